# revision 1
# baseline (speedup 1.0000x reference)
"""Bass/Trainium2 8-core SPMD kernel for nn_EpiEPMP (2xGCN -> 2xGAT -> BN/FC).

Graph-parallel, destination-partitioned strategy:
  - Nodes partitioned contiguously across 8 cores (2500 ab + 2500 ag each).
  - Per layer: local x@W on TensorE (feature-major lhsT), AllGather of
    per-node "table" rows to every core's HBM, then per-edge dma_gather of
    source rows (host pre-sorts edges by destination window), segment-reduced
    on TensorE with per-chunk one-hot selection matrices built by one fused
    tensor_scalar (is_equal * scale).
  - GAT attention: table rows are [1 | h | BIG+hs].  Per chunk
    T1 = (iota==dst_e)*(BIG+hs_e); T2 = T1 + (hd_j - BIG);
    U = max(exp(T2), exp(0.2*T2)) which equals exp(leaky_relu_0.2(hs+hd)) on
    one-hot positions and ~0 elsewhere; one matmul per chunk accumulates the
    numerator (cols 1..256) and softmax denominator (col 0) in PSUM.
  - BatchNorm: stats accumulated with ScalarE accum_out in transposed layout,
    AllReduced, applied as fused per-partition tensor_scalar x*A+B.
  - All index/padding/normalization planning on the host; the device program
    is fully static and identical on all 8 cores (SPMD).
"""

import sys

sys.path.insert(0, "/opt/trn_rl_repo")

import numpy as np
from concourse import bacc, mybir
from concourse.tile import TileContext
from concourse import library_config

P = 128
F = 256
CORES = 8
BIG = 150.0
EPS = 1e-5
I16_SPLIT = 32768
TABW = 320  # padded GAT table row (floats): [1 | h(256) | BIG+hs | pad]
NB = 10     # exp-batch size in chunks

F32 = mybir.dt.float32
I16 = mybir.dt.int16
AF = mybir.ActivationFunctionType
OP = mybir.AluOpType


# ----------------------------------------------------------------------------
# host-side planning
# ----------------------------------------------------------------------------

def _wrap_idx(idx):
    """[n] -> [128, n//16] int16; index i at partition i%16, slot i//16,
    replicated across the 8 Q7 cores (16-partition groups)."""
    n = len(idx)
    assert n % 16 == 0
    w = idx.reshape(n // 16, 16).T.astype(np.int16)
    return np.tile(w, (8, 1))


def _cols(arr):
    """[n] -> [128, n//128] f32; edge e at partition e%128, slot e//128."""
    n = len(arr)
    assert n % P == 0
    return np.ascontiguousarray(arr.reshape(n // P, P).T.astype(np.float32))


def _plan_agg(src, dst, coeff, n_loc, n_cores, split):
    """Destination-partitioned aggregation plan.  Returns (win_k, per_core):
    win_k[w] = [k_half0(, k_half1)] chunk counts, identical across cores;
    per_core[c] = dict(idx [128,S] i16, dstc [128,C] f32, coefc or None)."""
    owner = dst // n_loc
    loc = dst % n_loc
    n_win = -(-n_loc // P)
    halves = 2 if split is not None else 1

    win_of = loc // P
    order = np.lexsort((src, win_of, owner))
    so, lo, wo = src[order], loc[order], win_of[order]
    co = coeff[order] if coeff is not None else None
    key = owner[order] * n_win + wo
    starts = np.searchsorted(key, np.arange(n_cores * n_win), side="left")
    ends = np.searchsorted(key, np.arange(n_cores * n_win), side="right")

    buckets = {}
    for c in range(n_cores):
        for w in range(n_win):
            a, b = starts[c * n_win + w], ends[c * n_win + w]
            s_, l_ = so[a:b], lo[a:b]
            c_ = co[a:b] if co is not None else None
            if halves == 2:
                m = s_ < split
                buckets[c, w] = [
                    (s_[m], l_[m], None if c_ is None else c_[m]),
                    (s_[~m] - split, l_[~m], None if c_ is None else c_[~m])]
            else:
                buckets[c, w] = [(s_, l_, c_)]

    win_k = []
    for w in range(n_win):
        ks = []
        for h in range(halves):
            mx = max(len(buckets[c, w][h][0]) for c in range(n_cores))
            ks.append(-(-mx // P))
        if sum(ks) == 0:
            ks[0] = 1
        win_k.append(ks)

    per_core = []
    for c in range(n_cores):
        ip, dp, cp = [], [], []
        for w in range(n_win):
            for h in range(halves):
                k = win_k[w][h]
                if k == 0:
                    continue
                s_, l_, c_ = buckets[c, w][h]
                pad = k * P - len(s_)
                ip.append(_wrap_idx(np.concatenate([s_, np.zeros(pad, np.int64)])))
                dp.append(_cols(np.concatenate(
                    [(l_ % P).astype(np.float32), np.full(pad, -1.0, np.float32)])))
                if c_ is not None:
                    cp.append(_cols(np.concatenate(
                        [c_.astype(np.float32), np.zeros(pad, np.float32)])))
        per_core.append(dict(
            idx=np.concatenate(ip, axis=1),
            dstc=np.concatenate(dp, axis=1),
            coefc=(np.concatenate(cp, axis=1) if coeff is not None else None)))
    return win_k, per_core


def _gcn_edges(ei, n):
    src = np.concatenate([ei[0], np.arange(n)]).astype(np.int64)
    dst = np.concatenate([ei[1], np.arange(n)]).astype(np.int64)
    deg = np.bincount(dst, minlength=n).astype(np.float64)
    dinv = 1.0 / np.sqrt(np.maximum(deg, 1.0))
    return src, dst, (dinv[src] * dinv[dst]).astype(np.float32)


def build_host_plan(inputs, n_ab, n_ag, n_cores):
    nl_ab, nl_ag = n_ab // n_cores, n_ag // n_cores
    nl_g = nl_ab + nl_ag

    s_ab, d_ab, c_ab = _gcn_edges(np.asarray(inputs["edge_x_ab"]), n_ab)
    s_ag, d_ag, c_ag = _gcn_edges(np.asarray(inputs["edge_x_ag"]), n_ag)
    wk_ab, pc_ab = _plan_agg(s_ab, d_ab, c_ab, nl_ab, n_cores, None)
    wk_ag, pc_ag = _plan_agg(s_ag, d_ag, c_ag, nl_ag, n_cores, None)

    ed = np.asarray(inputs["edge_index_d"]).astype(np.int64)
    n_g = n_ab + n_ag
    sd = np.concatenate([ed[0], np.arange(n_g)])
    dd = np.concatenate([ed[1], np.arange(n_g)])

    def remap(g):
        isab = g < n_ab
        j = g - n_ab
        return np.where(isab, (g // nl_ab) * nl_g + g % nl_ab,
                        (j // nl_ag) * nl_g + nl_ab + j % nl_ag)

    split = I16_SPLIT if n_g > I16_SPLIT else None
    wk_g, pc_g = _plan_agg(remap(sd), remap(dd), None, nl_g, n_cores, split)

    f32 = lambda k: np.asarray(inputs[k], np.float32)
    W1 = np.concatenate([f32("W_gat"), (f32("W_gat") @ f32("a_src"))[:, None]], 1)
    W2 = np.concatenate([f32("W_gat2"), (f32("W_gat2") @ f32("a_src2"))[:, None]], 1)

    consts = dict(
        iota=np.broadcast_to(np.arange(P, dtype=np.float32), (P, P)).copy(),
        ident=np.eye(P, dtype=np.float32),
        ones_row=np.ones((1, P), np.float32),
        bgat_b=np.broadcast_to(f32("b_gat"), (P, F)).copy(),
        W_gcn_ab=f32("W_gcn").reshape(2, P, F).transpose(1, 0, 2),
        W_gcn_ag=f32("W_aggcn").reshape(2, P, F).transpose(1, 0, 2),
        W1=W1.reshape(2, P, F + 1).transpose(1, 0, 2),
        W2=W2.reshape(2, P, F + 1).transpose(1, 0, 2),
        wd1=(f32("W_gat") @ f32("a_dst")).reshape(2, P).T.reshape(P, 2, 1),
        wd2=(f32("W_gat2") @ f32("a_dst2")).reshape(2, P).T.reshape(P, 2, 1),
        g1c=f32("g1").reshape(2, P).T.copy(), be1c=f32("be1").reshape(2, P).T.copy(),
        agg1c=f32("ag_g1").reshape(2, P).T.copy(),
        agbe1c=f32("ag_be1").reshape(2, P).T.copy(),
        g2c=f32("g2").reshape(4, P).T.copy(), be2c=f32("be2").reshape(4, P).T.copy(),
        agg2c=f32("ag_g2").reshape(4, P).T.copy(),
        agbe2c=f32("ag_be2").reshape(4, P).T.copy(),
        wfc=f32("W_fc").reshape(4, P).T.copy(),
        wagfc=f32("W_agfc").reshape(4, P).T.copy(),
    )
    scalars = dict(bfc=float(np.asarray(inputs["b_fc"]).reshape(-1)[0]),
                   bagfc=float(np.asarray(inputs["b_agfc"]).reshape(-1)[0]),
                   n_bn=float(n_ab))
    assert n_ab == n_ag

    x_ab, x_ag = f32("x_ab"), f32("x_ag")
    in_maps = []
    for c in range(n_cores):
        m = dict(consts)
        m["xT_ab"] = np.ascontiguousarray(
            x_ab[c * nl_ab:(c + 1) * nl_ab].T.reshape(2, P, nl_ab).transpose(1, 0, 2))
        m["xT_ag"] = np.ascontiguousarray(
            x_ag[c * nl_ag:(c + 1) * nl_ag].T.reshape(2, P, nl_ag).transpose(1, 0, 2))
        for g, pc in (("gab", pc_ab), ("gag", pc_ag), ("gg", pc_g)):
            m[f"{g}_idx"] = pc[c]["idx"]
            m[f"{g}_dst"] = pc[c]["dstc"]
            if pc[c]["coefc"] is not None:
                m[f"{g}_cf"] = pc[c]["coefc"]
        in_maps.append(m)

    static = dict(n_ab=n_ab, n_ag=n_ag, nl_ab=nl_ab, nl_ag=nl_ag, nl_g=nl_g,
                  wk_ab=wk_ab, wk_ag=wk_ag, wk_g=wk_g, split=split,
                  scalars=scalars,
                  shapes={k: v.shape for k, v in in_maps[0].items()},
                  dtypes={k: str(v.dtype) for k, v in in_maps[0].items()})
    return static, in_maps


# ----------------------------------------------------------------------------
# bass program
# ----------------------------------------------------------------------------

def build_bass(st):
    nl_ab, nl_ag, nl_g = st["nl_ab"], st["nl_ag"], st["nl_g"]
    n_ab, n_ag = st["n_ab"], st["n_ag"]
    n_g = n_ab + n_ag
    sc = st["scalars"]

    kmax_gat = max(sum(ks) for ks in st["wk_g"])
    kmax_gcn = max(max(ks[0] for ks in st["wk_ab"]),
                   max(ks[0] for ks in st["wk_ag"]))

    nc = bacc.Bacc("TRN2", num_devices=CORES, target_bir_lowering=False)

    ins = {}
    for k, shp in st["shapes"].items():
        ins[k] = nc.declare_dram_parameter(
            k, list(shp), I16 if st["dtypes"][k] == "int16" else F32,
            isOutput=False)
    out_ab = nc.declare_dram_parameter("out_ab", [1, nl_ab], F32, isOutput=True)
    out_ag = nc.declare_dram_parameter("out_ag", [1, nl_ag], F32, isOutput=True)

    rg = [list(range(CORES))]

    with TileContext(nc) as tc:
        with (
            tc.tile_pool(name="dram", bufs=1, space="DRAM") as dr,
            tc.tile_pool(name="const", bufs=1) as cst,
            tc.tile_pool(name="xtreg", bufs=2) as xtp,
            tc.tile_pool(name="gath", bufs=2) as gpool,
            tc.tile_pool(name="strip", bufs=2) as spool,
            tc.tile_pool(name="work", bufs=2) as wrk,
            tc.tile_pool(name="small", bufs=4) as sm,
            tc.tile_pool(name="ps", bufs=2, space="PSUM") as pp,
        ):
            nc.gpsimd.load_library(library_config.mlp)

            # ---------------- DRAM scratch ----------------
            tab_ab_in = dr.tile([nl_ab, F], F32)
            tab_ag_in = dr.tile([nl_ag, F], F32)
            tab_ab = dr.tile([n_ab, F], F32, addr_space="Shared")
            tab_ag = dr.tile([n_ag, F], F32, addr_space="Shared")
            tab1_in = dr.tile([nl_g, TABW], F32)
            tab2_in = dr.tile([nl_g, TABW], F32)
            tab1 = dr.tile([n_g, TABW], F32, addr_space="Shared")
            tab2 = dr.tile([n_g, TABW], F32, addr_space="Shared")
            hd1_dr = dr.tile([1, nl_g], F32)
            hd2_dr = dr.tile([1, nl_g], F32)
            yt_ab_dr = dr.tile([P, 2, nl_ab], F32)
            yt_ag_dr = dr.tile([P, 2, nl_ag], F32)
            bn1_in = dr.tile([P, 8], F32)
            bn1_out = dr.tile([P, 8], F32, addr_space="Shared")
            bn2_in = dr.tile([P, 16], F32)
            bn2_out = dr.tile([P, 16], F32, addr_space="Shared")

            # ---------------- constants ----------------
            def load(k, pool=cst, tag=None):
                t = pool.tile(list(st["shapes"][k]),
                              I16 if st["dtypes"][k] == "int16" else F32,
                              name=k, tag=(tag or k))
                nc.sync.dma_start(out=t[...], in_=ins[k][...])
                return t

            iota_t = load("iota")
            ident_t = load("ident")
            ones_row_t = load("ones_row")
            bgat_t = load("bgat_b")
            Wab_t, Wag_t = load("W_gcn_ab"), load("W_gcn_ag")
            W1_t, W2_t = load("W1"), load("W2")
            wd1_t, wd2_t = load("wd1"), load("wd2")
            bn1cols = {k: load(k) for k in ("g1c", "be1c", "agg1c", "agbe1c")}
            bn2cols = {k: load(k) for k in ("g2c", "be2c", "agg2c", "agbe2c")}
            wfc_t, wagfc_t = load("wfc"), load("wagfc")
            xin_ab = load("xT_ab", xtp, tag="xtreg")
            xin_ag = load("xT_ag", xtp, tag="xtreg")
            # idx/dst/cf share rotating slots (phases are sequential)
            gidx, gdst, gcf = {}, {}, {}
            for g in ("gab", "gag", "gg"):
                gidx[g] = load(f"{g}_idx", tag="idxshare")
                gdst[g] = load(f"{g}_dst", tag="dstshare")
                if f"{g}_cf" in ins:
                    gcf[g] = load(f"{g}_cf", tag="cfshare")

            # ============ phase 1: GCN x@W -> table bounce ============
            def gcn_mm(xin, W_t, tab_in, n_loc):
                for t in range(-(-n_loc // P)):
                    m = min(P, n_loc - t * P)
                    pm = pp.tile([P, F + 1], F32, tag="bigps", space="PSUM")
                    for h in range(2):
                        nc.tensor.matmul(
                            out=pm[:m, :F], lhsT=xin[:, h, t * P:t * P + m],
                            rhs=W_t[:, h, :], start=(h == 0), stop=(h == 1))
                    sb = wrk.tile([P, F], F32, tag="mmsb")
                    nc.scalar.activation(out=sb[:m, :], in_=pm[:m, :F], func=AF.Copy)
                    nc.sync.dma_start(out=tab_in[t * P:t * P + m, :], in_=sb[:m, :])

            gcn_mm(xin_ab, Wab_t, tab_ab_in, nl_ab)
            gcn_mm(xin_ag, Wag_t, tab_ag_in, nl_ag)

            nc.gpsimd.collective_compute(
                "AllGather", OP.bypass, replica_groups=rg,
                ins=[tab_ab_in[...].opt()], outs=[tab_ab[...].opt()])
            nc.gpsimd.collective_compute(
                "AllGather", OP.bypass, replica_groups=rg,
                ins=[tab_ag_in[...].opt()], outs=[tab_ag[...].opt()])

            # ============ phase 3: GCN aggregation + BN1 stats ============
            bn_ab = xtp.tile([P, 2, nl_ab], F32, name="bn_ab", tag="xtreg")
            bn_ag = xtp.tile([P, 2, nl_ag], F32, name="bn_ag", tag="xtreg")
            bn1_sb = sm.tile([P, 8], F32, bufs=1)

            def gcn_agg(g, wk_list, tab, n_loc, bn_reg, col0):
                n_win = -(-n_loc // P)
                idx_off = 0
                ch_off = 0
                s_sum = spool.tile([P, 2 * n_win], F32, tag=f"st_{g}", bufs=1)
                s_sq = spool.tile([P, 2 * n_win], F32, tag=f"stq_{g}", bufs=1)
                for w in range(n_win):
                    m = min(P, n_loc - w * P)
                    k = wk_list[w][0]
                    gt = gpool.tile([P, k, F], F32, tag="gbuf")
                    for a in range(0, k, 8):
                        kk = min(8, k - a)
                        nc.gpsimd.dma_gather(
                            out_ap=gt[:, a:a + kk, :], in_ap=tab[...],
                            idxs_ap=gidx[g][:, idx_off + a * 8:
                                            idx_off + (a + kk) * 8],
                            num_idxs=kk * P, num_idxs_reg=kk * P, elem_size=F)
                    pm = pp.tile([P, F + 1], F32, tag="bigps", space="PSUM")
                    for c in range(k):
                        u = wrk.tile([P, P], F32, tag="usel", bufs=4)
                        nc.vector.tensor_scalar(
                            out=u[...], in0=iota_t[...],
                            scalar1=gdst[g][:, ch_off + c:ch_off + c + 1],
                            scalar2=gcf[g][:, ch_off + c:ch_off + c + 1],
                            op0=OP.is_equal, op1=OP.mult)
                        nc.tensor.matmul(out=pm[:, :F], lhsT=u[...],
                                         rhs=gt[:, c, :],
                                         start=(c == 0), stop=(c == k - 1))
                    idx_off += k * 8
                    ch_off += k
                    hsb = wrk.tile([P, F], F32, tag="drainsb")
                    nc.scalar.activation(out=hsb[...], in_=pm[:, :F], func=AF.Copy)
                    for h in range(2):
                        pt = pp.tile([P, P], F32, tag="trps", space="PSUM")
                        nc.tensor.transpose(
                            out=pt[...], in_=hsb[:, h * P:(h + 1) * P],
                            identity=ident_t[...])
                        nc.vector.tensor_copy(
                            out=bn_reg[:, h, w * P:w * P + m], in_=pt[:, :m])
                        hT = wrk.tile([P, P], F32, tag="htsb")
                        nc.scalar.activation(
                            out=hT[:, :m], in_=pt[:, :m], func=AF.Copy,
                            accum_out=s_sum[:, 2 * w + h:2 * w + h + 1])
                        nc.scalar.activation(
                            out=hT[:, :m], in_=pt[:, :m], func=AF.Square,
                            accum_out=s_sq[:, 2 * w + h:2 * w + h + 1])
                for h in range(2):
                    nc.scalar.activation(
                        out=s_sum[:, h::2], in_=s_sum[:, h::2], func=AF.Copy,
                        accum_out=bn1_sb[:, col0 + h:col0 + h + 1])
                    nc.scalar.activation(
                        out=s_sq[:, h::2], in_=s_sq[:, h::2], func=AF.Copy,
                        accum_out=bn1_sb[:, col0 + 2 + h:col0 + 3 + h])

            gcn_agg("gab", st["wk_ab"], tab_ab, nl_ab, bn_ab, 0)
            gcn_agg("gag", st["wk_ag"], tab_ag, nl_ag, bn_ag, 4)

            nc.sync.dma_start(out=bn1_in[...], in_=bn1_sb[...])
            nc.gpsimd.collective_compute(
                "AllReduce", OP.add, replica_groups=rg,
                ins=[bn1_in[...].opt()], outs=[bn1_out[...].opt()])
            bn1_red = sm.tile([P, 8], F32, bufs=1)
            nc.sync.dma_start(out=bn1_red[...], in_=bn1_out[...])

            # ============ phase 5: BN apply (+relu), transposed layout ======
            def bn_coeffs(sum_sl, sq_sl, gcol, becol, nf, tagp):
                mu = sm.tile([P, nf], F32, tag=tagp + "mu")
                nc.vector.tensor_scalar(out=mu[...], in0=sum_sl,
                                        scalar1=1.0 / sc["n_bn"], scalar2=None,
                                        op0=OP.mult)
                m2 = sm.tile([P, nf], F32, tag=tagp + "m2")
                nc.vector.tensor_scalar(out=m2[...], in0=sq_sl,
                                        scalar1=1.0 / sc["n_bn"], scalar2=None,
                                        op0=OP.mult)
                musq = sm.tile([P, nf], F32, tag=tagp + "musq")
                nc.scalar.activation(out=musq[...], in_=mu[...], func=AF.Square)
                var = sm.tile([P, nf], F32, tag=tagp + "var")
                nc.vector.tensor_tensor(out=var[...], in0=m2[...], in1=musq[...],
                                        op=OP.subtract)
                vep = sm.tile([P, nf], F32, tag=tagp + "vep")
                nc.vector.tensor_scalar(out=vep[...], in0=var[...],
                                        scalar1=EPS, scalar2=None, op0=OP.add)
                lnv = sm.tile([P, nf], F32, tag=tagp + "ln")
                nc.scalar.activation(out=lnv[...], in_=vep[...], func=AF.Ln)
                rsq = sm.tile([P, nf], F32, tag=tagp + "rsq")
                nc.scalar.activation(out=rsq[...], in_=lnv[...], func=AF.Exp,
                                     scale=-0.5)
                A = sm.tile([P, nf], F32, tag=tagp + "A")
                nc.vector.tensor_tensor(out=A[...], in0=gcol[...], in1=rsq[...],
                                        op=OP.mult)
                muA = sm.tile([P, nf], F32, tag=tagp + "muA")
                nc.vector.tensor_tensor(out=muA[...], in0=mu[...], in1=A[...],
                                        op=OP.mult)
                B = sm.tile([P, nf], F32, tag=tagp + "B")
                nc.vector.tensor_tensor(out=B[...], in0=becol[...], in1=muA[...],
                                        op=OP.subtract)
                return A, B

            for sum_sl, sq_sl, gk, bek, reg, ytd in (
                    (bn1_red[:, 0:2], bn1_red[:, 2:4], "g1c", "be1c", bn_ab, yt_ab_dr),
                    (bn1_red[:, 4:6], bn1_red[:, 6:8], "agg1c", "agbe1c", bn_ag, yt_ag_dr)):
                A, B = bn_coeffs(sum_sl, sq_sl, bn1cols[gk], bn1cols[bek], 2, "b1")
                for h in range(2):
                    nc.vector.tensor_scalar(
                        out=reg[:, h, :], in0=reg[:, h, :],
                        scalar1=A[:, h:h + 1], scalar2=B[:, h:h + 1],
                        op0=OP.mult, op1=OP.add)
                    nc.vector.tensor_scalar(
                        out=reg[:, h, :], in0=reg[:, h, :],
                        scalar1=0.0, scalar2=None, op0=OP.max)
                nc.sync.dma_start(out=ytd[...], in_=reg[...])

            # ============ phase 6/9: GAT x@W -> table + hd ============
            def gat_mm(regs, W_t, wd_t, tab_in, hd_dr):
                off = 0
                for reg, n_loc in regs:
                    for t in range(-(-n_loc // P)):
                        m = min(P, n_loc - t * P)
                        pm = pp.tile([P, F + 1], F32, tag="bigps", space="PSUM")
                        ph = pp.tile([1, 512], F32, tag="rowps", space="PSUM")
                        for h in range(2):
                            nc.tensor.matmul(
                                out=pm[:m, :], lhsT=reg[:, h, t * P:t * P + m],
                                rhs=W_t[:, h, :], start=(h == 0), stop=(h == 1))
                        for h in range(2):
                            nc.tensor.matmul(
                                out=ph[:1, :m], lhsT=wd_t[:, h, :],
                                rhs=reg[:, h, t * P:t * P + m],
                                start=(h == 0), stop=(h == 1))
                        sb = wrk.tile([P, TABW], F32, tag="tabsb")
                        nc.vector.memset(sb[...], 0.0)
                        nc.vector.memset(sb[:, 0:1], 1.0)
                        nc.scalar.activation(out=sb[:m, 1:F + 1], in_=pm[:m, 0:F],
                                             func=AF.Copy)
                        nc.vector.tensor_scalar(
                            out=sb[:m, F + 1:F + 2], in0=pm[:m, F:F + 1],
                            scalar1=BIG, scalar2=None, op0=OP.add)
                        hsb = sm.tile([1, P], F32, tag="hdsb")
                        nc.vector.tensor_scalar(
                            out=hsb[:, :m], in0=ph[:1, :m], scalar1=-BIG,
                            scalar2=None, op0=OP.add)
                        nc.sync.dma_start(
                            out=tab_in[off + t * P:off + t * P + m, :],
                            in_=sb[:m, :])
                        nc.sync.dma_start(
                            out=hd_dr[:, off + t * P:off + t * P + m],
                            in_=hsb[:, :m])
                    off += n_loc

            gat_mm([(bn_ab, nl_ab), (bn_ag, nl_ag)], W1_t, wd1_t, tab1_in, hd1_dr)
            nc.gpsimd.collective_compute(
                "AllGather", OP.bypass, replica_groups=rg,
                ins=[tab1_in[...].opt()], outs=[tab1[...].opt()])

            # ============ phase 8/11: GAT aggregation ============
            def gat_agg(wk_list, tab, hd_dr, dest_regs, relu_bias):
                n_win = len(wk_list)
                idx_off = 0
                ch_off = 0
                for w in range(n_win):
                    ks = wk_list[w]
                    ktot = sum(ks)
                    mw = min(P, nl_g - w * P)
                    gt = gpool.tile([P, kmax_gat, TABW], F32, tag="gbuf")
                    co = 0
                    for h, k in enumerate(ks):
                        if k == 0:
                            continue
                        src_ap = tab[...] if h == 0 else tab[I16_SPLIT:, :]
                        for a in range(0, k, 6):
                            kk = min(6, k - a)
                            nc.gpsimd.dma_gather(
                                out_ap=gt[:, co + a:co + a + kk, :],
                                in_ap=src_ap,
                                idxs_ap=gidx["gg"][:, idx_off + a * 8:
                                                   idx_off + (a + kk) * 8],
                                num_idxs=kk * P, num_idxs_reg=kk * P,
                                elem_size=TABW)
                        idx_off += k * 8
                        co += k
                    hdrow = sm.tile([1, P], F32, tag="hdrow")
                    nc.vector.memset(hdrow[...], -BIG)
                    nc.sync.dma_start(out=hdrow[:, :mw],
                                      in_=hd_dr[:, w * P:w * P + mw])
                    phd = pp.tile([P, P], F32, tag="hdbc", space="PSUM")
                    nc.tensor.matmul(out=phd[...], lhsT=ones_row_t[...],
                                     rhs=hdrow[...], start=True, stop=True)
                    pm = pp.tile([P, F + 1], F32, tag="bigps", space="PSUM")
                    done = 0
                    while done < ktot:
                        nb = min(NB, ktot - done)
                        t2 = spool.tile([P, NB * P], F32, tag="t2")
                        ust = spool.tile([P, NB * P], F32, tag="ustr")
                        for c in range(done, done + nb):
                            j = c - done
                            t1 = wrk.tile([P, P], F32, tag="usel", bufs=4)
                            nc.vector.tensor_scalar(
                                out=t1[...], in0=iota_t[...],
                                scalar1=gdst["gg"][:, ch_off + c:ch_off + c + 1],
                                scalar2=gt[:, c, F + 1:F + 2],
                                op0=OP.is_equal, op1=OP.mult)
                            nc.vector.tensor_tensor(
                                out=t2[:, j * P:(j + 1) * P], in0=t1[...],
                                in1=phd[...], op=OP.add)
                        nc.scalar.activation(out=ust[:, :nb * P],
                                             in_=t2[:, :nb * P], func=AF.Exp)
                        nc.scalar.activation(out=t2[:, :nb * P],
                                             in_=t2[:, :nb * P], func=AF.Exp,
                                             scale=0.2)
                        nc.vector.tensor_tensor(out=ust[:, :nb * P],
                                                in0=ust[:, :nb * P],
                                                in1=t2[:, :nb * P], op=OP.max)
                        for c in range(done, done + nb):
                            j = c - done
                            nc.tensor.matmul(
                                out=pm[...], lhsT=ust[:, j * P:(j + 1) * P],
                                rhs=gt[:, c, 0:F + 1],
                                start=(c == 0), stop=(c == ktot - 1))
                        done += nb
                    ch_off += ktot
                    den = sm.tile([P, 1], F32, tag="den")
                    nc.vector.tensor_scalar(out=den[...], in0=pm[:, 0:1],
                                            scalar1=1e-30, scalar2=None,
                                            op0=OP.add)
                    rcp = sm.tile([P, 1], F32, tag="rcp")
                    nc.vector.reciprocal(out=rcp[...], in_=den[...])
                    xo = wrk.tile([P, F], F32, tag="xo")
                    nc.vector.tensor_scalar(out=xo[...], in0=pm[:, 1:F + 1],
                                            scalar1=rcp[...], scalar2=None,
                                            op0=OP.mult)
                    if relu_bias:
                        nc.vector.tensor_tensor(out=xo[...], in0=xo[...],
                                                in1=bgat_t[...], op=OP.add)
                        nc.vector.tensor_scalar(out=xo[...], in0=xo[...],
                                                scalar1=0.0, scalar2=None,
                                                op0=OP.max)
                    base = w * P
                    for h in range(2):
                        pt = pp.tile([P, P], F32, tag="trps", space="PSUM")
                        nc.tensor.transpose(out=pt[...],
                                            in_=xo[:, h * P:(h + 1) * P],
                                            identity=ident_t[...])
                        for reg, pstart, plen in dest_regs:
                            a = max(base, pstart)
                            b = min(base + P, pstart + plen)
                            if a < b:
                                nc.vector.tensor_copy(
                                    out=reg[:, h, a - pstart:b - pstart],
                                    in_=pt[:, a - base:b - base])

            g2_ab = xtp.tile([P, 2, nl_ab], F32, name="g2_ab", tag="xtreg")
            g2_ag = xtp.tile([P, 2, nl_ag], F32, name="g2_ag", tag="xtreg")
            gat_agg(st["wk_g"], tab1, hd1_dr,
                    [(g2_ab, 0, nl_ab), (g2_ag, nl_ab, nl_ag)], relu_bias=True)

            gat_mm([(g2_ab, nl_ab), (g2_ag, nl_ag)], W2_t, wd2_t, tab2_in, hd2_dr)
            nc.gpsimd.collective_compute(
                "AllGather", OP.bypass, replica_groups=rg,
                ins=[tab2_in[...].opt()], outs=[tab2[...].opt()])

            x1_ab = xtp.tile([P, 2, nl_ab], F32, name="x1_ab", tag="xtreg")
            x1_ag = xtp.tile([P, 2, nl_ag], F32, name="x1_ag", tag="xtreg")
            gat_agg(st["wk_g"], tab2, hd2_dr,
                    [(x1_ab, 0, nl_ab), (x1_ag, nl_ab, nl_ag)], relu_bias=False)

            # ============ phase 12: BN2 + FC ============
            bn2_sb = sm.tile([P, 16], F32, bufs=1)
            for si, (x1reg, yt_dr, n_loc) in enumerate(
                    [(x1_ab, yt_ab_dr, nl_ab), (x1_ag, yt_ag_dr, nl_ag)]):
                for ft in range(4):
                    if ft < 2:
                        src = x1reg[:, ft, :]
                    else:
                        yt = wrk.tile([P, n_loc], F32, tag="ytld", bufs=2)
                        nc.sync.dma_start(out=yt[...], in_=yt_dr[:, ft - 2, :])
                        src = yt[...]
                    col = si * 8 + ft * 2
                    sqt = wrk.tile([P, n_loc], F32, tag="sq2", bufs=1)
                    nc.scalar.activation(out=sqt[...], in_=src, func=AF.Copy,
                                         accum_out=bn2_sb[:, col:col + 1])
                    nc.scalar.activation(out=sqt[...], in_=src, func=AF.Square,
                                         accum_out=bn2_sb[:, col + 1:col + 2])

            nc.sync.dma_start(out=bn2_in[...], in_=bn2_sb[...])
            nc.gpsimd.collective_compute(
                "AllReduce", OP.add, replica_groups=rg,
                ins=[bn2_in[...].opt()], outs=[bn2_out[...].opt()])
            bn2_red = sm.tile([P, 16], F32, bufs=1)
            nc.sync.dma_start(out=bn2_red[...], in_=bn2_out[...])

            for si, (x1reg, yt_dr, gk, bek, wt, bconst, outp, n_loc) in enumerate([
                    (x1_ab, yt_ab_dr, "g2c", "be2c", wfc_t, sc["bfc"], out_ab,
                     nl_ab),
                    (x1_ag, yt_ag_dr, "agg2c", "agbe2c", wagfc_t, sc["bagfc"],
                     out_ag, nl_ag)]):
                A, B = bn_coeffs(bn2_red[:, si * 8:si * 8 + 8:2],
                                 bn2_red[:, si * 8 + 1:si * 8 + 8:2],
                                 bn2cols[gk], bn2cols[bek], 4, "b2")
                ftiles = []
                for ft in range(4):
                    if ft < 2:
                        src = x1reg[:, ft, :]
                    else:
                        yt = wrk.tile([P, n_loc], F32, tag="ytld", bufs=2)
                        nc.sync.dma_start(out=yt[...], in_=yt_dr[:, ft - 2, :])
                        src = yt[...]
                    nc.vector.tensor_scalar(
                        out=src, in0=src,
                        scalar1=A[:, ft:ft + 1], scalar2=B[:, ft:ft + 1],
                        op0=OP.mult, op1=OP.add)
                    nc.vector.tensor_scalar(
                        out=src, in0=src,
                        scalar1=0.0, scalar2=None, op0=OP.max)
                    ftiles.append(src)
                for s0 in range(0, n_loc, 512):
                    m = min(512, n_loc - s0)
                    pf = pp.tile([1, 512], F32, tag="rowps", space="PSUM")
                    for ft in range(4):
                        nc.tensor.matmul(
                            out=pf[:1, :m], lhsT=wt[:, ft:ft + 1],
                            rhs=ftiles[ft][:, s0:s0 + m],
                            start=(ft == 0), stop=(ft == 3))
                    ob = sm.tile([1, 512], F32, tag="fcsb")
                    nc.vector.tensor_scalar(out=ob[:, :m], in0=pf[:1, :m],
                                            scalar1=bconst, scalar2=None,
                                            op0=OP.add)
                    nc.sync.dma_start(out=outp[:, s0:s0 + m], in_=ob[:, :m])

    nc.finalize()
    return nc


# ----------------------------------------------------------------------------
# runner
# ----------------------------------------------------------------------------

_CACHE = {}


def _run(inputs, n_ab, n_ag, trace=False, sim=False):
    static, in_maps = build_host_plan(inputs, n_ab, n_ag, CORES)
    key = (n_ab, n_ag,
           hash(np.asarray(inputs["edge_index_d"]).tobytes()) ^
           hash(np.asarray(inputs["edge_x_ab"]).tobytes()) ^
           hash(np.asarray(inputs["edge_x_ag"]).tobytes()) ^
           hash(repr(sorted(static["scalars"].items()))))
    if key not in _CACHE:
        _CACHE[key] = build_bass(static)
    nc = _CACHE[key]
    nl_ab, nl_ag = n_ab // CORES, n_ag // CORES

    if sim:
        from concourse import bass_interp
        s = bass_interp.MultiCoreSim(nc, CORES)
        for i in range(CORES):
            for k, v in in_maps[i].items():
                s.cores[i].tensor(k)[:] = v
        s.simulate()
        o_ab = np.concatenate(
            [s.cores[c].mem_tensor("out_ab").reshape(nl_ab, 1)
             for c in range(CORES)], 0)
        o_ag = np.concatenate(
            [s.cores[c].mem_tensor("out_ag").reshape(nl_ag, 1)
             for c in range(CORES)], 0)
        return (o_ab, o_ag), None

    import importlib.util
    try:
        spec = importlib.util.spec_from_file_location(
            "antenv.axon_hooks", "/opt/trn_rl_repo/antenv/axon_hooks.py")
        mod = importlib.util.module_from_spec(spec)
        spec.loader.exec_module(mod)
        sys.modules.setdefault("antenv.axon_hooks", mod)
    except Exception:
        pass
    from concourse.bass_utils import run_bass_kernel_spmd
    r = run_bass_kernel_spmd(nc, in_maps, core_ids=list(range(CORES)),
                             trace=trace)
    o_ab = np.concatenate(
        [r.results[c]["out_ab"].reshape(nl_ab, 1) for c in range(CORES)], 0)
    o_ag = np.concatenate(
        [r.results[c]["out_ag"].reshape(nl_ag, 1) for c in range(CORES)], 0)
    return (o_ab, o_ag), r


def kernel(**inputs):
    (o_ab, o_ag), _ = _run(inputs, 20000, 20000)
    return o_ab, o_ag



# revision 16
# speedup vs baseline: 1.1583x; 1.1583x over previous
"""Bass/Trainium2 8-core SPMD kernel for nn_EpiEPMP (2xGCN -> 2xGAT -> BN/FC).

Graph-parallel, destination-partitioned, bf16 data plane:
  - Nodes partitioned contiguously across 8 cores (2500 ab + 2500 ag each).
  - Per layer: local x@W on TensorE (bf16), AllGather of per-node bf16
    "table" rows to every core's HBM, then per-window dma_gather of
    source rows (host pre-sorts edges by destination window).
  - Scatter/segment-reduction on TensorE: host-built bf16 selection
    masks (coeff-at-onehot for GCN, {0,1} onehot for GAT) are DMA'd and
    used directly as matmul lhsT -- no on-device is_equal.
  - Self-loop edges use no gather indices: chunk 0 of every window is a
    plain contiguous DMA of the core's own table rows plus a diagonal
    host mask.
  - GAT attention per window: logits = mask*hs_bcast + hd_bcast (strip
    TensorTensor ops), leaky-relu via scalar_tensor_tensor max(x,0.2x),
    exp on ScalarE, then U = exp * mask; one matmul per chunk
    accumulates numerator (cols 1..256) and softmax denominator (col 0,
    table rows carry a leading 1) in fp32 PSUM.
  - BatchNorm: stats accumulated with ScalarE accum_out in transposed
    layout (fp32), AllReduced, applied as fused per-partition
    tensor_scalar x*A+B.
  - All index/padding/normalization planning on the host; the device
    program is fully static and identical on all 8 cores (SPMD).
"""

import sys

sys.path.insert(0, "/opt/trn_rl_repo")

import numpy as np
import ml_dtypes
from concourse import bacc, mybir
from concourse.tile import TileContext
from concourse import library_config

BF = np.float16

P = 128
F = 256
CORES = 8
EPS = 1e-5
I16_SPLIT = 32768
TABW = 384  # padded GAT table row (bf16): [1 | h(256) | hs | pad] -> 768B

F32 = mybir.dt.float32
BF16 = mybir.dt.float16  # 2-byte data plane dtype (fp16: finer mantissa)
I16 = mybir.dt.int16
AF = mybir.ActivationFunctionType
OP = mybir.AluOpType


# ----------------------------------------------------------------------------
# host-side planning
# ----------------------------------------------------------------------------

def _wrap_idx(idx):
    """[n] -> [128, n//16] int16; index i at partition i%16, slot i//16,
    replicated across the 8 Q7 cores (16-partition groups)."""
    n = len(idx)
    assert n % 16 == 0
    w = idx.reshape(n // 16, 16).T.astype(np.int16)
    return np.tile(w, (8, 1))


def _plan_agg(src, dst, coeff, selfc, n_loc, n_cores, split):
    """Destination-partitioned aggregation plan with host-built masks.

    src/dst: REAL edges only (no self loops), global ids.
    coeff[e] per-edge value or None (-> 1.0).
    selfc[n] per-node self-loop value or None (-> 1.0).
    Returns (win_k, per_core):
      win_k[w] = [k_half0(, k_half1)] real-edge chunk counts (identical
        across cores; chunk 0 = self loops is implicit and not counted);
      per_core[c] = dict(idx [128, sum_k*8] i16,
                         mask [128, (n_win + sum_k)*128] bf16).
    """
    owner = dst // n_loc
    loc = dst % n_loc
    n_win = -(-n_loc // P)
    halves = 2 if split is not None else 1

    win_of = loc // P
    order = np.lexsort((src, win_of, owner))
    so, lo, wo = src[order], loc[order], win_of[order]
    co = coeff[order] if coeff is not None else None
    key = owner[order] * n_win + wo
    starts = np.searchsorted(key, np.arange(n_cores * n_win), side="left")
    ends = np.searchsorted(key, np.arange(n_cores * n_win), side="right")

    buckets = {}
    for c in range(n_cores):
        for w in range(n_win):
            a, b = starts[c * n_win + w], ends[c * n_win + w]
            s_, l_ = so[a:b], lo[a:b]
            c_ = co[a:b] if co is not None else None
            if halves == 2:
                m = s_ < split
                buckets[c, w] = [
                    (s_[m], l_[m], None if c_ is None else c_[m]),
                    (s_[~m] - split, l_[~m], None if c_ is None else c_[~m])]
            else:
                buckets[c, w] = [(s_, l_, c_)]

    win_k = []
    for w in range(n_win):
        ks = []
        for h in range(halves):
            mx = max(len(buckets[c, w][h][0]) for c in range(n_cores))
            ks.append(-(-mx // P))
        win_k.append(ks)

    per_core = []
    for c in range(n_cores):
        ip, mp = [], []
        for w in range(n_win):
            m = min(P, n_loc - w * P)
            # chunk 0: self loops (no indices; diagonal mask)
            sm = np.zeros((P, P), np.float32)
            gl = c * n_loc + w * P + np.arange(m)
            sm[np.arange(m), np.arange(m)] = \
                1.0 if selfc is None else selfc[gl]
            mp.append(sm)
            for h in range(halves):
                k = win_k[w][h]
                if k == 0:
                    continue
                s_, l_, c_ = buckets[c, w][h]
                ne = len(s_)
                pad = k * P - ne
                ip.append(_wrap_idx(np.concatenate(
                    [s_, np.zeros(pad, np.int64)])))
                mk = np.zeros((P, k * P), np.float32)
                e = np.arange(ne)
                mk[e % P, (e // P) * P + (l_ % P)] = \
                    1.0 if c_ is None else c_
                mp.append(mk)
        per_core.append(dict(
            idx=(np.concatenate(ip, axis=1) if ip else
                 np.zeros((P, 8), np.int16)),
            mask=np.concatenate(mp, axis=1).astype(BF)))
    return win_k, per_core


def _gcn_edges(ei, n):
    """Real edges + per-node self coeff for GCN normalization."""
    src = ei[0].astype(np.int64)
    dst = ei[1].astype(np.int64)
    deg = np.bincount(dst, minlength=n).astype(np.float64) + 1.0  # self loop
    dinv = 1.0 / np.sqrt(deg)
    return src, dst, (dinv[src] * dinv[dst]).astype(np.float32), \
        (dinv * dinv).astype(np.float32)


def build_host_plan(inputs, n_ab, n_ag, n_cores):
    nl_ab, nl_ag = n_ab // n_cores, n_ag // n_cores
    nl_g = nl_ab + nl_ag

    s_ab, d_ab, c_ab, sc_ab = _gcn_edges(np.asarray(inputs["edge_x_ab"]), n_ab)
    s_ag, d_ag, c_ag, sc_ag = _gcn_edges(np.asarray(inputs["edge_x_ag"]), n_ag)
    wk_ab, pc_ab = _plan_agg(s_ab, d_ab, c_ab, sc_ab, nl_ab, n_cores, None)
    wk_ag, pc_ag = _plan_agg(s_ag, d_ag, c_ag, sc_ag, nl_ag, n_cores, None)

    ed = np.asarray(inputs["edge_index_d"]).astype(np.int64)
    n_g = n_ab + n_ag

    def remap(g):
        isab = g < n_ab
        j = g - n_ab
        return np.where(isab, (g // nl_ab) * nl_g + g % nl_ab,
                        (j // nl_ag) * nl_g + nl_ab + j % nl_ag)

    split = I16_SPLIT if n_g > I16_SPLIT else None
    wk_g, pc_g = _plan_agg(remap(ed[0]), remap(ed[1]), None, None,
                           nl_g, n_cores, split)

    f32 = lambda k: np.asarray(inputs[k], np.float32)
    bf = lambda a: np.ascontiguousarray(a).astype(BF)
    W1 = np.concatenate([f32("W_gat"), (f32("W_gat") @ f32("a_src"))[:, None]], 1)
    W2 = np.concatenate([f32("W_gat2"), (f32("W_gat2") @ f32("a_src2"))[:, None]], 1)

    consts = dict(
        ident=bf(np.eye(P, dtype=np.float32)),
        ones_row=bf(np.ones((1, P), np.float32)),
        bgat_b=bf(np.broadcast_to(f32("b_gat"), (P, F))),
        W_gcn_ab=bf(f32("W_gcn").reshape(2, P, F).transpose(1, 0, 2)),
        W_gcn_ag=bf(f32("W_aggcn").reshape(2, P, F).transpose(1, 0, 2)),
        W1=bf(W1.reshape(2, P, F + 1).transpose(1, 0, 2)),
        W2=bf(W2.reshape(2, P, F + 1).transpose(1, 0, 2)),
        wd1=bf((f32("W_gat") @ f32("a_dst")).reshape(2, P).T.reshape(P, 2, 1)),
        wd2=bf((f32("W_gat2") @ f32("a_dst2")).reshape(2, P).T.reshape(P, 2, 1)),
        g1c=f32("g1").reshape(2, P).T.copy(), be1c=f32("be1").reshape(2, P).T.copy(),
        agg1c=f32("ag_g1").reshape(2, P).T.copy(),
        agbe1c=f32("ag_be1").reshape(2, P).T.copy(),
        g2c=f32("g2").reshape(4, P).T.copy(), be2c=f32("be2").reshape(4, P).T.copy(),
        agg2c=f32("ag_g2").reshape(4, P).T.copy(),
        agbe2c=f32("ag_be2").reshape(4, P).T.copy(),
        wfc=bf(f32("W_fc").reshape(4, P).T),
        wagfc=bf(f32("W_agfc").reshape(4, P).T),
    )
    scalars = dict(bfc=float(np.asarray(inputs["b_fc"]).reshape(-1)[0]),
                   bagfc=float(np.asarray(inputs["b_agfc"]).reshape(-1)[0]),
                   n_bn=float(n_ab))
    assert n_ab == n_ag

    x_ab, x_ag = f32("x_ab"), f32("x_ag")
    in_maps = []
    for c in range(n_cores):
        m = dict(consts)
        m["xT_ab"] = bf(x_ab[c * nl_ab:(c + 1) * nl_ab]
                        .T.reshape(2, P, nl_ab).transpose(1, 0, 2))
        m["xT_ag"] = bf(x_ag[c * nl_ag:(c + 1) * nl_ag]
                        .T.reshape(2, P, nl_ag).transpose(1, 0, 2))
        for g, pc in (("gab", pc_ab), ("gag", pc_ag), ("gg", pc_g)):
            m[f"{g}_idx"] = pc[c]["idx"]
            m[f"{g}_mask"] = pc[c]["mask"]
        in_maps.append(m)

    static = dict(n_ab=n_ab, n_ag=n_ag, nl_ab=nl_ab, nl_ag=nl_ag, nl_g=nl_g,
                  wk_ab=wk_ab, wk_ag=wk_ag, wk_g=wk_g, split=split,
                  scalars=scalars,
                  shapes={k: v.shape for k, v in in_maps[0].items()},
                  dtypes={k: str(v.dtype) for k, v in in_maps[0].items()})
    return static, in_maps


# ----------------------------------------------------------------------------
# bass program
# ----------------------------------------------------------------------------

def build_bass(st):
    nl_ab, nl_ag, nl_g = st["nl_ab"], st["nl_ag"], st["nl_g"]
    n_ab, n_ag = st["n_ab"], st["n_ag"]
    n_g = n_ab + n_ag
    sc = st["scalars"]
    split = st["split"]

    kmax_gat = max(1 + sum(ks) for ks in st["wk_g"])
    kmax_gcn = max(max(1 + ks[0] for ks in st["wk_ab"]),
                   max(1 + ks[0] for ks in st["wk_ag"]))

    nc = bacc.Bacc("TRN2", num_devices=CORES, target_bir_lowering=False)

    def dt_of(k):
        s = st["dtypes"][k]
        return I16 if s == "int16" else (BF16 if s == "float16" else F32)

    ins = {}
    for k, shp in st["shapes"].items():
        ins[k] = nc.declare_dram_parameter(k, list(shp), dt_of(k),
                                           isOutput=False)
    out_ab = nc.declare_dram_parameter("out_ab", [1, nl_ab], F32, isOutput=True)
    out_ag = nc.declare_dram_parameter("out_ag", [1, nl_ag], F32, isOutput=True)

    rg = [list(range(CORES))]

    with TileContext(nc) as tc:
        with (
            tc.tile_pool(name="dram", bufs=1, space="DRAM") as dr,
            tc.tile_pool(name="const", bufs=1) as cst,
            tc.tile_pool(name="xtreg", bufs=2) as xtp,
            tc.tile_pool(name="gath", bufs=2) as gpool,
            tc.tile_pool(name="mask", bufs=2) as mpool,
            tc.tile_pool(name="strip", bufs=2) as spool,
            tc.tile_pool(name="work", bufs=2) as wrk,
            tc.tile_pool(name="small", bufs=4) as sm,
            tc.tile_pool(name="ps", bufs=2, space="PSUM") as pp,
        ):
            nc.gpsimd.load_library(library_config.mlp)

            # ---------------- DRAM scratch ----------------
            tab_ab_in = dr.tile([nl_ab, F], BF16)
            tab_ag_in = dr.tile([nl_ag, F], BF16)
            tab_ab = dr.tile([n_ab, F], BF16, addr_space="Shared")
            tab_ag = dr.tile([n_ag, F], BF16, addr_space="Shared")
            tab1_in = dr.tile([nl_g, TABW], BF16)
            tab2_in = dr.tile([nl_g, TABW], BF16)
            tab1 = dr.tile([n_g, TABW], BF16, addr_space="Shared")
            tab2 = dr.tile([n_g, TABW], BF16, addr_space="Shared")
            hd1_dr = dr.tile([1, nl_g], BF16)
            hd2_dr = dr.tile([1, nl_g], BF16)
            yt_ab_dr = dr.tile([P, 2, nl_ab], BF16)
            yt_ag_dr = dr.tile([P, 2, nl_ag], BF16)
            bn1_in = dr.tile([P, 8], F32)
            bn1_out = dr.tile([P, 8], F32, addr_space="Shared")
            bn2_in = dr.tile([P, 16], F32)
            bn2_out = dr.tile([P, 16], F32, addr_space="Shared")

            # ---------------- constants ----------------
            def load(k, pool=cst, tag=None):
                t = pool.tile(list(st["shapes"][k]), dt_of(k),
                              name=k, tag=(tag or k))
                nc.sync.dma_start(out=t[...], in_=ins[k][...])
                return t

            ident_t = load("ident")
            ones_row_t = load("ones_row")
            bgat_t = load("bgat_b")
            Wab_t, Wag_t = load("W_gcn_ab"), load("W_gcn_ag")
            W1_t, W2_t = load("W1"), load("W2")
            wd1_t, wd2_t = load("wd1"), load("wd2")
            bn1cols = {k: load(k) for k in ("g1c", "be1c", "agg1c", "agbe1c")}
            bn2cols = {k: load(k) for k in ("g2c", "be2c", "agg2c", "agbe2c")}
            wfc_t, wagfc_t = load("wfc"), load("wagfc")
            xin_ab = load("xT_ab", xtp, tag="xtreg")
            xin_ag = load("xT_ag", xtp, tag="xtreg")
            gidx = {g: load(f"{g}_idx", tag="idxshare")
                    for g in ("gab", "gag", "gg")}

            # ============ phase 1: GCN x@W -> table bounce ============
            def gcn_mm(xin, W_t, tab_in, n_loc):
                for t in range(-(-n_loc // P)):
                    m = min(P, n_loc - t * P)
                    pm = pp.tile([P, F + 1], F32, tag="bigps", space="PSUM")
                    for h in range(2):
                        nc.tensor.matmul(
                            out=pm[:m, :F], lhsT=xin[:, h, t * P:t * P + m],
                            rhs=W_t[:, h, :], start=(h == 0), stop=(h == 1))
                    sb = wrk.tile([P, F], BF16, tag="mmsb")
                    nc.scalar.activation(out=sb[:m, :], in_=pm[:m, :F], func=AF.Copy)
                    nc.sync.dma_start(out=tab_in[t * P:t * P + m, :], in_=sb[:m, :])

            gcn_mm(xin_ab, Wab_t, tab_ab_in, nl_ab)
            nc.gpsimd.collective_compute(
                "AllGather", OP.bypass, replica_groups=rg,
                ins=[tab_ab_in[...].opt()], outs=[tab_ab[...].opt()])
            gcn_mm(xin_ag, Wag_t, tab_ag_in, nl_ag)
            nc.gpsimd.collective_compute(
                "AllGather", OP.bypass, replica_groups=rg,
                ins=[tab_ag_in[...].opt()], outs=[tab_ag[...].opt()])

            # ============ phase 3: GCN aggregation + BN1 stats ============
            bn_ab = xtp.tile([P, 2, nl_ab], BF16, name="bn_ab", tag="xtreg")
            bn_ag = xtp.tile([P, 2, nl_ag], BF16, name="bn_ag", tag="xtreg")
            bn1_sb = sm.tile([P, 8], F32, bufs=1)

            def gcn_agg(g, wk_list, tab, tab_in, n_loc, bn_reg, col0):
                n_win = -(-n_loc // P)
                idx_off = 0
                mcol_off = 0
                s_sum = spool.tile([P, 2 * n_win], F32, tag=f"st_{g}", bufs=1)
                s_sq = spool.tile([P, 2 * n_win], F32, tag=f"stq_{g}", bufs=1)
                for w in range(n_win):
                    m = min(P, n_loc - w * P)
                    k = wk_list[w][0]
                    K = 1 + k
                    gt = gpool.tile([P, kmax_gcn, F], BF16, tag="gbuf")
                    if m < P:
                        nc.vector.memset(gt[:, 0, :], 0.0)
                    nc.sync.dma_start(
                        out=gt[:m, 0, :],
                        in_=tab_in[w * P:w * P + m, :])
                    for a in range(0, k, 8):
                        kk = min(8, k - a)
                        nc.gpsimd.dma_gather(
                            out_ap=gt[:, 1 + a:1 + a + kk, :], in_ap=tab[...],
                            idxs_ap=gidx[g][:, idx_off + a * 8:
                                            idx_off + (a + kk) * 8],
                            num_idxs=kk * P, num_idxs_reg=kk * P, elem_size=F)
                    idx_off += k * 8
                    mk = mpool.tile([P, kmax_gcn, P], BF16, tag="mkbuf")
                    nc.sync.dma_start(
                        out=mk[:, :K, :],
                        in_=ins[f"{g}_mask"][:, mcol_off:mcol_off + K * P])
                    mcol_off += K * P
                    pm = pp.tile([P, F + 1], F32, tag="bigps", space="PSUM")
                    for c in range(K):
                        nc.tensor.matmul(out=pm[:, :F], lhsT=mk[:, c, :],
                                         rhs=gt[:, c, :],
                                         start=(c == 0), stop=(c == K - 1))
                    hsb = wrk.tile([P, F], BF16, tag="drainsb")
                    nc.scalar.activation(out=hsb[...], in_=pm[:, :F], func=AF.Copy)
                    for h in range(2):
                        pt = pp.tile([P, P], BF16, tag="trps", space="PSUM")
                        nc.tensor.transpose(
                            out=pt[...], in_=hsb[:, h * P:(h + 1) * P],
                            identity=ident_t[...])
                        nc.scalar.activation(
                            out=bn_reg[:, h, w * P:w * P + m], in_=pt[:, :m],
                            func=AF.Copy,
                            accum_out=s_sum[:, 2 * w + h:2 * w + h + 1])
                        hT = wrk.tile([P, P], F32, tag="htsb")
                        nc.scalar.activation(
                            out=hT[:, :m], in_=pt[:, :m], func=AF.Square,
                            accum_out=s_sq[:, 2 * w + h:2 * w + h + 1])
                for h in range(2):
                    nc.scalar.activation(
                        out=s_sum[:, h::2], in_=s_sum[:, h::2], func=AF.Copy,
                        accum_out=bn1_sb[:, col0 + h:col0 + h + 1])
                    nc.scalar.activation(
                        out=s_sq[:, h::2], in_=s_sq[:, h::2], func=AF.Copy,
                        accum_out=bn1_sb[:, col0 + 2 + h:col0 + 3 + h])

            gcn_agg("gab", st["wk_ab"], tab_ab, tab_ab_in, nl_ab, bn_ab, 0)
            gcn_agg("gag", st["wk_ag"], tab_ag, tab_ag_in, nl_ag, bn_ag, 4)

            nc.sync.dma_start(out=bn1_in[...], in_=bn1_sb[...])
            nc.gpsimd.collective_compute(
                "AllReduce", OP.add, replica_groups=rg,
                ins=[bn1_in[...].opt()], outs=[bn1_out[...].opt()])
            bn1_red = sm.tile([P, 8], F32, bufs=1)
            nc.sync.dma_start(out=bn1_red[...], in_=bn1_out[...])

            # ============ phase 5: BN apply (+relu), transposed layout ======
            def bn_coeffs(sum_sl, sq_sl, gcol, becol, nf, tagp):
                mu = sm.tile([P, nf], F32, tag=tagp + "mu")
                nc.vector.tensor_scalar(out=mu[...], in0=sum_sl,
                                        scalar1=1.0 / sc["n_bn"], scalar2=None,
                                        op0=OP.mult)
                m2 = sm.tile([P, nf], F32, tag=tagp + "m2")
                nc.vector.tensor_scalar(out=m2[...], in0=sq_sl,
                                        scalar1=1.0 / sc["n_bn"], scalar2=None,
                                        op0=OP.mult)
                musq = sm.tile([P, nf], F32, tag=tagp + "musq")
                nc.scalar.activation(out=musq[...], in_=mu[...], func=AF.Square)
                var = sm.tile([P, nf], F32, tag=tagp + "var")
                nc.vector.tensor_tensor(out=var[...], in0=m2[...], in1=musq[...],
                                        op=OP.subtract)
                vep = sm.tile([P, nf], F32, tag=tagp + "vep")
                nc.vector.tensor_scalar(out=vep[...], in0=var[...],
                                        scalar1=EPS, scalar2=None, op0=OP.add)
                lnv = sm.tile([P, nf], F32, tag=tagp + "ln")
                nc.scalar.activation(out=lnv[...], in_=vep[...], func=AF.Ln)
                rsq = sm.tile([P, nf], F32, tag=tagp + "rsq")
                nc.scalar.activation(out=rsq[...], in_=lnv[...], func=AF.Exp,
                                     scale=-0.5)
                A = sm.tile([P, nf], F32, tag=tagp + "A")
                nc.vector.tensor_tensor(out=A[...], in0=gcol[...], in1=rsq[...],
                                        op=OP.mult)
                muA = sm.tile([P, nf], F32, tag=tagp + "muA")
                nc.vector.tensor_tensor(out=muA[...], in0=mu[...], in1=A[...],
                                        op=OP.mult)
                B = sm.tile([P, nf], F32, tag=tagp + "B")
                nc.vector.tensor_tensor(out=B[...], in0=becol[...], in1=muA[...],
                                        op=OP.subtract)
                return A, B

            for sum_sl, sq_sl, gk, bek, reg, ytd in (
                    (bn1_red[:, 0:2], bn1_red[:, 2:4], "g1c", "be1c", bn_ab, yt_ab_dr),
                    (bn1_red[:, 4:6], bn1_red[:, 6:8], "agg1c", "agbe1c", bn_ag, yt_ag_dr)):
                A, B = bn_coeffs(sum_sl, sq_sl, bn1cols[gk], bn1cols[bek], 2, "b1")
                for h in range(2):
                    nc.vector.tensor_scalar(
                        out=reg[:, h, :], in0=reg[:, h, :],
                        scalar1=A[:, h:h + 1], scalar2=B[:, h:h + 1],
                        op0=OP.mult, op1=OP.add)
                    nc.vector.tensor_scalar(
                        out=reg[:, h, :], in0=reg[:, h, :],
                        scalar1=0.0, scalar2=None, op0=OP.max)
                nc.sync.dma_start(out=ytd[...], in_=reg[...])

            # ============ phase 6/9: GAT x@W -> table + hd ============
            def gat_mm(regs, W_t, wd_t, tab_in, hd_dr):
                off = 0
                for reg, n_loc in regs:
                    for t in range(-(-n_loc // P)):
                        m = min(P, n_loc - t * P)
                        pm = pp.tile([P, F + 1], F32, tag="bigps", space="PSUM")
                        ph = pp.tile([1, 512], F32, tag="rowps", space="PSUM")
                        for h in range(2):
                            nc.tensor.matmul(
                                out=pm[:m, :], lhsT=reg[:, h, t * P:t * P + m],
                                rhs=W_t[:, h, :], start=(h == 0), stop=(h == 1))
                        for h in range(2):
                            nc.tensor.matmul(
                                out=ph[:1, :m], lhsT=wd_t[:, h, :],
                                rhs=reg[:, h, t * P:t * P + m],
                                start=(h == 0), stop=(h == 1))
                        sb = wrk.tile([P, TABW], BF16, tag="tabsb")
                        nc.vector.memset(sb[:, 0:1], 1.0)
                        nc.vector.memset(sb[:, F + 2:], 0.0)
                        nc.scalar.activation(out=sb[:m, 1:F + 2],
                                             in_=pm[:m, 0:F + 1], func=AF.Copy)
                        hsb = sm.tile([1, P], BF16, tag="hdsb")
                        nc.vector.tensor_copy(out=hsb[:, :m], in_=ph[:1, :m])
                        nc.sync.dma_start(
                            out=tab_in[off + t * P:off + t * P + m, :],
                            in_=sb[:m, :])
                        nc.sync.dma_start(
                            out=hd_dr[:, off + t * P:off + t * P + m],
                            in_=hsb[:, :m])
                    off += n_loc

            gat_mm([(bn_ab, nl_ab), (bn_ag, nl_ag)], W1_t, wd1_t, tab1_in, hd1_dr)
            nc.gpsimd.collective_compute(
                "AllGather", OP.bypass, replica_groups=rg,
                ins=[tab1_in[...].opt()], outs=[tab1[...].opt()])

            # ============ phase 8/11: GAT aggregation ============
            def gat_agg(wk_list, tab, tab_in, hd_dr, dest_regs, relu_bias):
                n_win = len(wk_list)
                idx_off = 0
                mcol_off = 0
                for w in range(n_win):
                    ks = wk_list[w]
                    K = 1 + sum(ks)
                    mw = min(P, nl_g - w * P)
                    gt = gpool.tile([P, kmax_gat, TABW], BF16, tag="gbuf")
                    if mw < P:
                        nc.vector.memset(gt[:, 0, :], 0.0)
                    nc.sync.dma_start(
                        out=gt[:mw, 0, :],
                        in_=tab_in[w * P:w * P + mw, :])
                    co = 1
                    for h, k in enumerate(ks):
                        if k == 0:
                            continue
                        src_ap = tab[...] if h == 0 else tab[I16_SPLIT:, :]
                        for a in range(0, k, 8):
                            kk = min(8, k - a)
                            nc.gpsimd.dma_gather(
                                out_ap=gt[:, co + a:co + a + kk, :],
                                in_ap=src_ap,
                                idxs_ap=gidx["gg"][:, idx_off + a * 8:
                                                   idx_off + (a + kk) * 8],
                                num_idxs=kk * P, num_idxs_reg=kk * P,
                                elem_size=TABW)
                        idx_off += k * 8
                        co += k
                    mk = mpool.tile([P, kmax_gat, P], BF16, tag="mkbuf")
                    nc.sync.dma_start(
                        out=mk[:, :K, :],
                        in_=ins["gg_mask"][:, mcol_off:mcol_off + K * P])
                    mcol_off += K * P
                    # hd broadcast for this window
                    hdrow = sm.tile([1, P], BF16, tag="hdrow")
                    nc.vector.memset(hdrow[...], 0.0)
                    nc.sync.dma_start(out=hdrow[:, :mw],
                                      in_=hd_dr[:, w * P:w * P + mw])
                    phd = pp.tile([P, P], F32, tag="hdbc", space="PSUM")
                    nc.tensor.matmul(out=phd[...], lhsT=ones_row_t[...],
                                     rhs=hdrow[...], start=True, stop=True)
                    phd_sb = sm.tile([P, P], BF16, tag="phdsb")
                    nc.vector.tensor_copy(out=phd_sb[...], in_=phd[...])
                    # logits strip: t2 = mask * hs_bcast + hd_bcast
                    t2 = spool.tile([P, kmax_gat, P], BF16, tag="t2")
                    nc.vector.tensor_tensor(
                        out=t2[:, :K, :], in0=mk[:, :K, :],
                        in1=gt[:, :K, F + 1:F + 2].broadcast_to([P, K, P]),
                        op=OP.mult)
                    nc.vector.tensor_tensor(
                        out=t2[:, :K, :], in0=t2[:, :K, :],
                        in1=phd_sb[...].unsqueeze(1).broadcast_to([P, K, P]),
                        op=OP.add)
                    # leaky relu (max(x, 0.2x)) then exp, then mask
                    nc.vector.scalar_tensor_tensor(
                        out=t2[:, :K, :], in0=t2[:, :K, :], scalar=0.2,
                        in1=t2[:, :K, :], op0=OP.mult, op1=OP.max)
                    nc.scalar.activation(out=t2[:, :K, :], in_=t2[:, :K, :],
                                         func=AF.Exp)
                    ust = spool.tile([P, kmax_gat, P], BF16, tag="ustr")
                    nc.vector.tensor_tensor(
                        out=ust[:, :K, :], in0=t2[:, :K, :], in1=mk[:, :K, :],
                        op=OP.mult)
                    pm = pp.tile([P, F + 1], F32, tag="bigps", space="PSUM")
                    for c in range(K):
                        nc.tensor.matmul(
                            out=pm[...], lhsT=ust[:, c, :],
                            rhs=gt[:, c, 0:F + 1],
                            start=(c == 0), stop=(c == K - 1))
                    den = sm.tile([P, 1], F32, tag="den")
                    nc.vector.tensor_scalar(out=den[...], in0=pm[:, 0:1],
                                            scalar1=1e-30, scalar2=None,
                                            op0=OP.add)
                    rcp = sm.tile([P, 1], F32, tag="rcp")
                    nc.vector.reciprocal(out=rcp[...], in_=den[...])
                    xo = wrk.tile([P, F], BF16, tag="xo")
                    nc.vector.tensor_scalar(out=xo[...], in0=pm[:, 1:F + 1],
                                            scalar1=rcp[...], scalar2=None,
                                            op0=OP.mult)
                    if relu_bias:
                        nc.vector.tensor_tensor(out=xo[...], in0=xo[...],
                                                in1=bgat_t[...], op=OP.add)
                        nc.vector.tensor_scalar(out=xo[...], in0=xo[...],
                                                scalar1=0.0, scalar2=None,
                                                op0=OP.max)
                    base = w * P
                    for h in range(2):
                        pt = pp.tile([P, P], BF16, tag="trps", space="PSUM")
                        nc.tensor.transpose(out=pt[...],
                                            in_=xo[:, h * P:(h + 1) * P],
                                            identity=ident_t[...])
                        for reg, pstart, plen in dest_regs:
                            a = max(base, pstart)
                            b = min(base + P, pstart + plen)
                            if a < b:
                                nc.vector.tensor_copy(
                                    out=reg[:, h, a - pstart:b - pstart],
                                    in_=pt[:, a - base:b - base])

            g2_ab = xtp.tile([P, 2, nl_ab], BF16, name="g2_ab", tag="xtreg")
            g2_ag = xtp.tile([P, 2, nl_ag], BF16, name="g2_ag", tag="xtreg")
            gat_agg(st["wk_g"], tab1, tab1_in, hd1_dr,
                    [(g2_ab, 0, nl_ab), (g2_ag, nl_ab, nl_ag)], relu_bias=True)

            gat_mm([(g2_ab, nl_ab), (g2_ag, nl_ag)], W2_t, wd2_t, tab2_in, hd2_dr)
            nc.gpsimd.collective_compute(
                "AllGather", OP.bypass, replica_groups=rg,
                ins=[tab2_in[...].opt()], outs=[tab2[...].opt()])

            x1_ab = xtp.tile([P, 2, nl_ab], BF16, name="x1_ab", tag="xtreg")
            x1_ag = xtp.tile([P, 2, nl_ag], BF16, name="x1_ag", tag="xtreg")
            gat_agg(st["wk_g"], tab2, tab2_in, hd2_dr,
                    [(x1_ab, 0, nl_ab), (x1_ag, nl_ab, nl_ag)], relu_bias=False)

            # ============ phase 12: BN2 + FC ============
            bn2_sb = sm.tile([P, 16], F32, bufs=1)
            for si, (x1reg, yt_dr, n_loc) in enumerate(
                    [(x1_ab, yt_ab_dr, nl_ab), (x1_ag, yt_ag_dr, nl_ag)]):
                for ft in range(4):
                    if ft < 2:
                        src = x1reg[:, ft, :]
                    else:
                        yt = wrk.tile([P, n_loc], BF16, tag="ytld", bufs=2)
                        nc.sync.dma_start(out=yt[...], in_=yt_dr[:, ft - 2, :])
                        src = yt[...]
                    col = si * 8 + ft * 2
                    sqt = wrk.tile([P, n_loc], F32, tag="sq2", bufs=1)
                    nc.scalar.activation(out=sqt[...], in_=src, func=AF.Copy,
                                         accum_out=bn2_sb[:, col:col + 1])
                    nc.scalar.activation(out=sqt[...], in_=src, func=AF.Square,
                                         accum_out=bn2_sb[:, col + 1:col + 2])

            nc.sync.dma_start(out=bn2_in[...], in_=bn2_sb[...])
            nc.gpsimd.collective_compute(
                "AllReduce", OP.add, replica_groups=rg,
                ins=[bn2_in[...].opt()], outs=[bn2_out[...].opt()])
            bn2_red = sm.tile([P, 16], F32, bufs=1)
            nc.sync.dma_start(out=bn2_red[...], in_=bn2_out[...])

            for si, (x1reg, yt_dr, gk, bek, wt, bconst, outp, n_loc) in enumerate([
                    (x1_ab, yt_ab_dr, "g2c", "be2c", wfc_t, sc["bfc"], out_ab,
                     nl_ab),
                    (x1_ag, yt_ag_dr, "agg2c", "agbe2c", wagfc_t, sc["bagfc"],
                     out_ag, nl_ag)]):
                A, B = bn_coeffs(bn2_red[:, si * 8:si * 8 + 8:2],
                                 bn2_red[:, si * 8 + 1:si * 8 + 8:2],
                                 bn2cols[gk], bn2cols[bek], 4, "b2")
                ftiles = []
                for ft in range(4):
                    if ft < 2:
                        src = x1reg[:, ft, :]
                    else:
                        yt = wrk.tile([P, n_loc], BF16, tag="ytld2", bufs=2)
                        nc.sync.dma_start(out=yt[...], in_=yt_dr[:, ft - 2, :])
                        src = yt[...]
                    nc.vector.tensor_scalar(
                        out=src, in0=src,
                        scalar1=A[:, ft:ft + 1], scalar2=B[:, ft:ft + 1],
                        op0=OP.mult, op1=OP.add)
                    nc.vector.tensor_scalar(
                        out=src, in0=src,
                        scalar1=0.0, scalar2=None, op0=OP.max)
                    ftiles.append(src)
                for s0 in range(0, n_loc, 512):
                    m = min(512, n_loc - s0)
                    pf = pp.tile([1, 512], F32, tag="rowps", space="PSUM")
                    for ft in range(4):
                        nc.tensor.matmul(
                            out=pf[:1, :m], lhsT=wt[:, ft:ft + 1],
                            rhs=ftiles[ft][:, s0:s0 + m],
                            start=(ft == 0), stop=(ft == 3))
                    ob = sm.tile([1, 512], F32, tag="fcsb")
                    nc.vector.tensor_scalar(out=ob[:, :m], in0=pf[:1, :m],
                                            scalar1=bconst, scalar2=None,
                                            op0=OP.add)
                    nc.sync.dma_start(out=outp[:, s0:s0 + m], in_=ob[:, :m])

    nc.finalize()
    return nc


# ----------------------------------------------------------------------------
# runner
# ----------------------------------------------------------------------------

_CACHE = {}


def _run(inputs, n_ab, n_ag, trace=False, sim=False):
    static, in_maps = build_host_plan(inputs, n_ab, n_ag, CORES)
    key = (n_ab, n_ag,
           hash(np.asarray(inputs["edge_index_d"]).tobytes()) ^
           hash(np.asarray(inputs["edge_x_ab"]).tobytes()) ^
           hash(np.asarray(inputs["edge_x_ag"]).tobytes()) ^
           hash(repr(sorted(static["scalars"].items()))))
    if key not in _CACHE:
        _CACHE[key] = build_bass(static)
    nc = _CACHE[key]
    nl_ab, nl_ag = n_ab // CORES, n_ag // CORES

    if sim:
        from concourse import bass_interp
        s = bass_interp.MultiCoreSim(nc, CORES)
        for i in range(CORES):
            for k, v in in_maps[i].items():
                s.cores[i].tensor(k)[:] = v
        s.simulate()
        o_ab = np.concatenate(
            [s.cores[c].mem_tensor("out_ab").reshape(nl_ab, 1)
             for c in range(CORES)], 0)
        o_ag = np.concatenate(
            [s.cores[c].mem_tensor("out_ag").reshape(nl_ag, 1)
             for c in range(CORES)], 0)
        return (o_ab, o_ag), None

    from concourse.bass_utils import run_bass_kernel_spmd
    r = run_bass_kernel_spmd(nc, in_maps, core_ids=list(range(CORES)),
                             trace=trace)
    o_ab = np.concatenate(
        [r.results[c]["out_ab"].reshape(nl_ab, 1) for c in range(CORES)], 0)
    o_ag = np.concatenate(
        [r.results[c]["out_ag"].reshape(nl_ag, 1) for c in range(CORES)], 0)
    return (o_ab, o_ag), r


def kernel(**inputs):
    (o_ab, o_ag), _ = _run(inputs, 20000, 20000)
    return o_ab, o_ag


# revision 17
# speedup vs baseline: 1.3285x; 1.1469x over previous
"""Bass/Trainium2 8-core SPMD kernel for nn_EpiEPMP (2xGCN -> 2xGAT -> BN/FC).

Graph-parallel, destination-partitioned, bf16 data plane:
  - Nodes partitioned contiguously across 8 cores (2500 ab + 2500 ag each).
  - Per layer: local x@W on TensorE (bf16), AllGather of per-node bf16
    "table" rows to every core's HBM, then per-window dma_gather of
    source rows (host pre-sorts edges by destination window).
  - Scatter/segment-reduction on TensorE: host-built bf16 selection
    masks (coeff-at-onehot for GCN, {0,1} onehot for GAT) are DMA'd and
    used directly as matmul lhsT -- no on-device is_equal.
  - Self-loop edges use no gather indices: chunk 0 of every window is a
    plain contiguous DMA of the core's own table rows plus a diagonal
    host mask.
  - GAT attention per window: logits = mask*hs_bcast + hd_bcast (strip
    TensorTensor ops), leaky-relu via scalar_tensor_tensor max(x,0.2x),
    exp on ScalarE, then U = exp * mask; one matmul per chunk
    accumulates numerator (cols 1..256) and softmax denominator (col 0,
    table rows carry a leading 1) in fp32 PSUM.
  - BatchNorm: stats accumulated with ScalarE accum_out in transposed
    layout (fp32), AllReduced, applied as fused per-partition
    tensor_scalar x*A+B.
  - All index/padding/normalization planning on the host; the device
    program is fully static and identical on all 8 cores (SPMD).
"""

import sys

sys.path.insert(0, "/opt/trn_rl_repo")

import numpy as np
import ml_dtypes
from concourse import bacc, mybir
from concourse.tile import TileContext
from concourse import library_config

BF = np.float16

P = 128
F = 256
CORES = 8
EPS = 1e-5
I16_SPLIT = 32768
TABW = 384  # padded GAT table row (bf16): [1 | h(256) | hs | pad] -> 768B

F32 = mybir.dt.float32
BF16 = mybir.dt.float16  # 2-byte data plane dtype (fp16: finer mantissa)
I16 = mybir.dt.int16
AF = mybir.ActivationFunctionType
OP = mybir.AluOpType


# ----------------------------------------------------------------------------
# host-side planning
# ----------------------------------------------------------------------------

def _wrap_idx(idx):
    """[n] -> [128, n//16] int16; index i at partition i%16, slot i//16,
    replicated across the 8 Q7 cores (16-partition groups)."""
    n = len(idx)
    assert n % 16 == 0
    w = idx.reshape(n // 16, 16).T.astype(np.int16)
    return np.tile(w, (8, 1))


def _plan_agg(src, dst, coeff, selfc, n_loc, n_cores, split):
    """Destination-partitioned aggregation plan with host-built masks.

    src/dst: REAL edges only (no self loops), global ids.
    coeff[e] per-edge value or None (-> 1.0).
    selfc[n] per-node self-loop value or None (-> 1.0).
    Returns (win_k, per_core):
      win_k[w] = [k_half0(, k_half1)] real-edge chunk counts (identical
        across cores; chunk 0 = self loops is implicit and not counted);
      per_core[c] = dict(idx [128, sum_k*8] i16,
                         mask [128, (n_win + sum_k)*128] bf16).
    """
    owner = dst // n_loc
    loc = dst % n_loc
    n_win = -(-n_loc // P)
    halves = 2 if split is not None else 1

    win_of = loc // P
    order = np.lexsort((src, win_of, owner))
    so, lo, wo = src[order], loc[order], win_of[order]
    co = coeff[order] if coeff is not None else None
    key = owner[order] * n_win + wo
    starts = np.searchsorted(key, np.arange(n_cores * n_win), side="left")
    ends = np.searchsorted(key, np.arange(n_cores * n_win), side="right")

    buckets = {}
    for c in range(n_cores):
        for w in range(n_win):
            a, b = starts[c * n_win + w], ends[c * n_win + w]
            s_, l_ = so[a:b], lo[a:b]
            c_ = co[a:b] if co is not None else None
            if halves == 2:
                m = s_ < split
                buckets[c, w] = [
                    (s_[m], l_[m], None if c_ is None else c_[m]),
                    (s_[~m] - split, l_[~m], None if c_ is None else c_[~m])]
            else:
                buckets[c, w] = [(s_, l_, c_)]

    win_k = []
    for w in range(n_win):
        ks = []
        for h in range(halves):
            mx = max(len(buckets[c, w][h][0]) for c in range(n_cores))
            ks.append(-(-mx // P))
        win_k.append(ks)

    per_core = []
    for c in range(n_cores):
        ip, mp = [], []
        for w in range(n_win):
            m = min(P, n_loc - w * P)
            # chunk 0: self loops (no indices; diagonal mask)
            sm = np.zeros((P, P), np.float32)
            gl = c * n_loc + w * P + np.arange(m)
            sm[np.arange(m), np.arange(m)] = \
                1.0 if selfc is None else selfc[gl]
            mp.append(sm)
            for h in range(halves):
                k = win_k[w][h]
                if k == 0:
                    continue
                s_, l_, c_ = buckets[c, w][h]
                ne = len(s_)
                pad = k * P - ne
                ip.append(_wrap_idx(np.concatenate(
                    [s_, np.zeros(pad, np.int64)])))
                mk = np.zeros((P, k * P), np.float32)
                e = np.arange(ne)
                mk[e % P, (e // P) * P + (l_ % P)] = \
                    1.0 if c_ is None else c_
                mp.append(mk)
        per_core.append(dict(
            idx=(np.concatenate(ip, axis=1) if ip else
                 np.zeros((P, 8), np.int16)),
            mask=np.concatenate(mp, axis=1).astype(BF)))
    return win_k, per_core


def _gcn_edges(ei, n):
    """Real edges + per-node self coeff for GCN normalization."""
    src = ei[0].astype(np.int64)
    dst = ei[1].astype(np.int64)
    deg = np.bincount(dst, minlength=n).astype(np.float64) + 1.0  # self loop
    dinv = 1.0 / np.sqrt(deg)
    return src, dst, (dinv[src] * dinv[dst]).astype(np.float32), \
        (dinv * dinv).astype(np.float32)


def build_host_plan(inputs, n_ab, n_ag, n_cores):
    nl_ab, nl_ag = n_ab // n_cores, n_ag // n_cores
    nl_g = nl_ab + nl_ag

    s_ab, d_ab, c_ab, sc_ab = _gcn_edges(np.asarray(inputs["edge_x_ab"]), n_ab)
    s_ag, d_ag, c_ag, sc_ag = _gcn_edges(np.asarray(inputs["edge_x_ag"]), n_ag)
    wk_ab, pc_ab = _plan_agg(s_ab, d_ab, c_ab, sc_ab, nl_ab, n_cores, None)
    wk_ag, pc_ag = _plan_agg(s_ag, d_ag, c_ag, sc_ag, nl_ag, n_cores, None)

    ed = np.asarray(inputs["edge_index_d"]).astype(np.int64)
    n_g = n_ab + n_ag

    def remap(g):
        isab = g < n_ab
        j = g - n_ab
        return np.where(isab, (g // nl_ab) * nl_g + g % nl_ab,
                        (j // nl_ag) * nl_g + nl_ab + j % nl_ag)

    split = I16_SPLIT if n_g > I16_SPLIT else None
    wk_g, pc_g = _plan_agg(remap(ed[0]), remap(ed[1]), None, None,
                           nl_g, n_cores, split)

    f32 = lambda k: np.asarray(inputs[k], np.float32)
    bf = lambda a: np.ascontiguousarray(a).astype(BF)
    W1 = np.concatenate([f32("W_gat"), (f32("W_gat") @ f32("a_src"))[:, None]], 1)
    W2 = np.concatenate([f32("W_gat2"), (f32("W_gat2") @ f32("a_src2"))[:, None]], 1)

    consts = dict(
        ident=bf(np.eye(P, dtype=np.float32)),
        ones_row=bf(np.ones((1, P), np.float32)),
        bgat_b=bf(np.broadcast_to(f32("b_gat"), (P, F))),
        W_gcn_ab=bf(f32("W_gcn").reshape(2, P, F).transpose(1, 0, 2)),
        W_gcn_ag=bf(f32("W_aggcn").reshape(2, P, F).transpose(1, 0, 2)),
        W1=bf(W1.reshape(2, P, F + 1).transpose(1, 0, 2)),
        W2=bf(W2.reshape(2, P, F + 1).transpose(1, 0, 2)),
        wd1=bf((f32("W_gat") @ f32("a_dst")).reshape(2, P).T.reshape(P, 2, 1)),
        wd2=bf((f32("W_gat2") @ f32("a_dst2")).reshape(2, P).T.reshape(P, 2, 1)),
        g1c=f32("g1").reshape(2, P).T.copy(), be1c=f32("be1").reshape(2, P).T.copy(),
        agg1c=f32("ag_g1").reshape(2, P).T.copy(),
        agbe1c=f32("ag_be1").reshape(2, P).T.copy(),
        g2c=f32("g2").reshape(4, P).T.copy(), be2c=f32("be2").reshape(4, P).T.copy(),
        agg2c=f32("ag_g2").reshape(4, P).T.copy(),
        agbe2c=f32("ag_be2").reshape(4, P).T.copy(),
        wfc=bf(f32("W_fc").reshape(4, P).T),
        wagfc=bf(f32("W_agfc").reshape(4, P).T),
    )
    scalars = dict(bfc=float(np.asarray(inputs["b_fc"]).reshape(-1)[0]),
                   bagfc=float(np.asarray(inputs["b_agfc"]).reshape(-1)[0]),
                   n_bn=float(n_ab))
    assert n_ab == n_ag

    x_ab, x_ag = f32("x_ab"), f32("x_ag")
    in_maps = []
    for c in range(n_cores):
        m = dict(consts)
        m["xT_ab"] = bf(x_ab[c * nl_ab:(c + 1) * nl_ab]
                        .T.reshape(2, P, nl_ab).transpose(1, 0, 2))
        m["xT_ag"] = bf(x_ag[c * nl_ag:(c + 1) * nl_ag]
                        .T.reshape(2, P, nl_ag).transpose(1, 0, 2))
        for g, pc in (("gab", pc_ab), ("gag", pc_ag), ("gg", pc_g)):
            m[f"{g}_idx"] = pc[c]["idx"]
            m[f"{g}_mask"] = pc[c]["mask"]
        in_maps.append(m)

    static = dict(n_ab=n_ab, n_ag=n_ag, nl_ab=nl_ab, nl_ag=nl_ag, nl_g=nl_g,
                  wk_ab=wk_ab, wk_ag=wk_ag, wk_g=wk_g, split=split,
                  scalars=scalars,
                  shapes={k: v.shape for k, v in in_maps[0].items()},
                  dtypes={k: str(v.dtype) for k, v in in_maps[0].items()})
    return static, in_maps


# ----------------------------------------------------------------------------
# bass program
# ----------------------------------------------------------------------------

def build_bass(st):
    nl_ab, nl_ag, nl_g = st["nl_ab"], st["nl_ag"], st["nl_g"]
    n_ab, n_ag = st["n_ab"], st["n_ag"]
    n_g = n_ab + n_ag
    sc = st["scalars"]
    split = st["split"]

    kmax_gat = max(1 + sum(ks) for ks in st["wk_g"])
    kmax_gcn = max(max(1 + ks[0] for ks in st["wk_ab"]),
                   max(1 + ks[0] for ks in st["wk_ag"]))

    nc = bacc.Bacc("TRN2", num_devices=CORES, target_bir_lowering=False)

    def dt_of(k):
        s = st["dtypes"][k]
        return I16 if s == "int16" else (BF16 if s == "float16" else F32)

    ins = {}
    for k, shp in st["shapes"].items():
        ins[k] = nc.declare_dram_parameter(k, list(shp), dt_of(k),
                                           isOutput=False)
    out_ab = nc.declare_dram_parameter("out_ab", [1, nl_ab], F32, isOutput=True)
    out_ag = nc.declare_dram_parameter("out_ag", [1, nl_ag], F32, isOutput=True)

    rg = [list(range(CORES))]

    with TileContext(nc) as tc:
        with (
            tc.tile_pool(name="dram", bufs=1, space="DRAM") as dr,
            tc.tile_pool(name="const", bufs=1) as cst,
            tc.tile_pool(name="xtreg", bufs=2) as xtp,
            tc.tile_pool(name="gath", bufs=3) as gpool,
            tc.tile_pool(name="mask", bufs=3) as mpool,
            tc.tile_pool(name="strip", bufs=2) as spool,
            tc.tile_pool(name="work", bufs=2) as wrk,
            tc.tile_pool(name="small", bufs=4) as sm,
            tc.tile_pool(name="ps", bufs=2, space="PSUM") as pp,
        ):
            nc.gpsimd.load_library(library_config.mlp)

            # ---------------- DRAM scratch ----------------
            tab_ab_in = dr.tile([nl_ab, F], BF16)
            tab_ag_in = dr.tile([nl_ag, F], BF16)
            tab_ab = dr.tile([n_ab, F], BF16, addr_space="Shared")
            tab_ag = dr.tile([n_ag, F], BF16, addr_space="Shared")
            tab1_in = dr.tile([nl_g, TABW], BF16)
            tab2_in = dr.tile([nl_g, TABW], BF16)
            tab1 = dr.tile([n_g, TABW], BF16, addr_space="Shared")
            tab2 = dr.tile([n_g, TABW], BF16, addr_space="Shared")
            hd1_dr = dr.tile([1, nl_g], BF16)
            hd2_dr = dr.tile([1, nl_g], BF16)
            yt_ab_dr = dr.tile([P, 2, nl_ab], BF16)
            yt_ag_dr = dr.tile([P, 2, nl_ag], BF16)
            bn1_in = dr.tile([P, 8], F32)
            bn1_out = dr.tile([P, 8], F32, addr_space="Shared")
            bn2_in = dr.tile([P, 16], F32)
            bn2_out = dr.tile([P, 16], F32, addr_space="Shared")

            # ---------------- constants ----------------
            def load(k, pool=cst, tag=None):
                t = pool.tile(list(st["shapes"][k]), dt_of(k),
                              name=k, tag=(tag or k))
                nc.sync.dma_start(out=t[...], in_=ins[k][...])
                return t

            ident_t = load("ident")
            ones_row_t = load("ones_row")
            bgat_t = load("bgat_b")
            Wab_t, Wag_t = load("W_gcn_ab"), load("W_gcn_ag")
            W1_t, W2_t = load("W1"), load("W2")
            wd1_t, wd2_t = load("wd1"), load("wd2")
            bn1cols = {k: load(k) for k in ("g1c", "be1c", "agg1c", "agbe1c")}
            bn2cols = {k: load(k) for k in ("g2c", "be2c", "agg2c", "agbe2c")}
            wfc_t, wagfc_t = load("wfc"), load("wagfc")
            xin_ab = load("xT_ab", xtp, tag="xtreg")
            xin_ag = load("xT_ag", xtp, tag="xtreg")
            gidx = {g: load(f"{g}_idx", tag="idxshare")
                    for g in ("gab", "gag", "gg")}

            # ============ phase 1: GCN x@W -> table bounce ============
            def gcn_mm(xin, W_t, tab_in, n_loc):
                for t in range(-(-n_loc // P)):
                    m = min(P, n_loc - t * P)
                    pm = pp.tile([P, F + 1], F32, tag="bigps", space="PSUM")
                    for h in range(2):
                        nc.tensor.matmul(
                            out=pm[:m, :F], lhsT=xin[:, h, t * P:t * P + m],
                            rhs=W_t[:, h, :], start=(h == 0), stop=(h == 1))
                    sb = wrk.tile([P, F], BF16, tag="mmsb")
                    nc.scalar.activation(out=sb[:m, :], in_=pm[:m, :F], func=AF.Copy)
                    nc.sync.dma_start(out=tab_in[t * P:t * P + m, :], in_=sb[:m, :])

            gcn_mm(xin_ab, Wab_t, tab_ab_in, nl_ab)
            nc.gpsimd.collective_compute(
                "AllGather", OP.bypass, replica_groups=rg,
                ins=[tab_ab_in[...].opt()], outs=[tab_ab[...].opt()])
            gcn_mm(xin_ag, Wag_t, tab_ag_in, nl_ag)
            nc.gpsimd.collective_compute(
                "AllGather", OP.bypass, replica_groups=rg,
                ins=[tab_ag_in[...].opt()], outs=[tab_ag[...].opt()])

            # ============ phase 3: GCN aggregation + BN1 stats ============
            bn_ab = xtp.tile([P, 2, nl_ab], BF16, name="bn_ab", tag="xtreg")
            bn_ag = xtp.tile([P, 2, nl_ag], BF16, name="bn_ag", tag="xtreg")
            bn1_sb = sm.tile([P, 8], F32, bufs=1)

            def gcn_agg(g, wk_list, tab, tab_in, n_loc, bn_reg, col0):
                n_win = -(-n_loc // P)
                idx_off = 0
                mcol_off = 0
                s_sum = spool.tile([P, 2 * n_win], F32, tag=f"st_{g}", bufs=1)
                s_sq = spool.tile([P, 2 * n_win], F32, tag=f"stq_{g}", bufs=1)
                for w in range(n_win):
                    m = min(P, n_loc - w * P)
                    k = wk_list[w][0]
                    K = 1 + k
                    gt = gpool.tile([P, kmax_gcn, F], BF16, tag="gbuf")
                    if m < P:
                        nc.vector.memset(gt[:, 0, :], 0.0)
                    nc.sync.dma_start(
                        out=gt[:m, 0, :],
                        in_=tab_in[w * P:w * P + m, :])
                    for a in range(0, k, 8):
                        kk = min(8, k - a)
                        nc.gpsimd.dma_gather(
                            out_ap=gt[:, 1 + a:1 + a + kk, :], in_ap=tab[...],
                            idxs_ap=gidx[g][:, idx_off + a * 8:
                                            idx_off + (a + kk) * 8],
                            num_idxs=kk * P, num_idxs_reg=kk * P, elem_size=F)
                    idx_off += k * 8
                    mk = mpool.tile([P, kmax_gcn, P], BF16, tag="mkbuf")
                    nc.sync.dma_start(
                        out=mk[:, :K, :],
                        in_=ins[f"{g}_mask"][:, mcol_off:mcol_off + K * P])
                    mcol_off += K * P
                    pm = pp.tile([P, F + 1], F32, tag="bigps", space="PSUM")
                    for c in range(K):
                        nc.tensor.matmul(out=pm[:, :F], lhsT=mk[:, c, :],
                                         rhs=gt[:, c, :],
                                         start=(c == 0), stop=(c == K - 1))
                    hsb = wrk.tile([P, F], BF16, tag="drainsb")
                    nc.scalar.activation(out=hsb[...], in_=pm[:, :F], func=AF.Copy)
                    for h in range(2):
                        pt = pp.tile([P, P], BF16, tag="trps", space="PSUM")
                        nc.tensor.transpose(
                            out=pt[...], in_=hsb[:, h * P:(h + 1) * P],
                            identity=ident_t[...])
                        nc.scalar.activation(
                            out=bn_reg[:, h, w * P:w * P + m], in_=pt[:, :m],
                            func=AF.Copy,
                            accum_out=s_sum[:, 2 * w + h:2 * w + h + 1])
                        hT = wrk.tile([P, P], F32, tag="htsb")
                        nc.scalar.activation(
                            out=hT[:, :m], in_=pt[:, :m], func=AF.Square,
                            accum_out=s_sq[:, 2 * w + h:2 * w + h + 1])
                for h in range(2):
                    nc.scalar.activation(
                        out=s_sum[:, h::2], in_=s_sum[:, h::2], func=AF.Copy,
                        accum_out=bn1_sb[:, col0 + h:col0 + h + 1])
                    nc.scalar.activation(
                        out=s_sq[:, h::2], in_=s_sq[:, h::2], func=AF.Copy,
                        accum_out=bn1_sb[:, col0 + 2 + h:col0 + 3 + h])

            gcn_agg("gab", st["wk_ab"], tab_ab, tab_ab_in, nl_ab, bn_ab, 0)
            gcn_agg("gag", st["wk_ag"], tab_ag, tab_ag_in, nl_ag, bn_ag, 4)

            nc.sync.dma_start(out=bn1_in[...], in_=bn1_sb[...])
            nc.gpsimd.collective_compute(
                "AllReduce", OP.add, replica_groups=rg,
                ins=[bn1_in[...].opt()], outs=[bn1_out[...].opt()])
            bn1_red = sm.tile([P, 8], F32, bufs=1)
            nc.sync.dma_start(out=bn1_red[...], in_=bn1_out[...])

            # ============ phase 5: BN apply (+relu), transposed layout ======
            def bn_coeffs(sum_sl, sq_sl, gcol, becol, nf, tagp):
                mu = sm.tile([P, nf], F32, tag=tagp + "mu")
                nc.vector.tensor_scalar(out=mu[...], in0=sum_sl,
                                        scalar1=1.0 / sc["n_bn"], scalar2=None,
                                        op0=OP.mult)
                m2 = sm.tile([P, nf], F32, tag=tagp + "m2")
                nc.vector.tensor_scalar(out=m2[...], in0=sq_sl,
                                        scalar1=1.0 / sc["n_bn"], scalar2=None,
                                        op0=OP.mult)
                musq = sm.tile([P, nf], F32, tag=tagp + "musq")
                nc.scalar.activation(out=musq[...], in_=mu[...], func=AF.Square)
                var = sm.tile([P, nf], F32, tag=tagp + "var")
                nc.vector.tensor_tensor(out=var[...], in0=m2[...], in1=musq[...],
                                        op=OP.subtract)
                vep = sm.tile([P, nf], F32, tag=tagp + "vep")
                nc.vector.tensor_scalar(out=vep[...], in0=var[...],
                                        scalar1=EPS, scalar2=None, op0=OP.add)
                lnv = sm.tile([P, nf], F32, tag=tagp + "ln")
                nc.scalar.activation(out=lnv[...], in_=vep[...], func=AF.Ln)
                rsq = sm.tile([P, nf], F32, tag=tagp + "rsq")
                nc.scalar.activation(out=rsq[...], in_=lnv[...], func=AF.Exp,
                                     scale=-0.5)
                A = sm.tile([P, nf], F32, tag=tagp + "A")
                nc.vector.tensor_tensor(out=A[...], in0=gcol[...], in1=rsq[...],
                                        op=OP.mult)
                muA = sm.tile([P, nf], F32, tag=tagp + "muA")
                nc.vector.tensor_tensor(out=muA[...], in0=mu[...], in1=A[...],
                                        op=OP.mult)
                B = sm.tile([P, nf], F32, tag=tagp + "B")
                nc.vector.tensor_tensor(out=B[...], in0=becol[...], in1=muA[...],
                                        op=OP.subtract)
                return A, B

            for sum_sl, sq_sl, gk, bek, reg, ytd in (
                    (bn1_red[:, 0:2], bn1_red[:, 2:4], "g1c", "be1c", bn_ab, yt_ab_dr),
                    (bn1_red[:, 4:6], bn1_red[:, 6:8], "agg1c", "agbe1c", bn_ag, yt_ag_dr)):
                A, B = bn_coeffs(sum_sl, sq_sl, bn1cols[gk], bn1cols[bek], 2, "b1")
                for h in range(2):
                    nc.vector.tensor_scalar(
                        out=reg[:, h, :], in0=reg[:, h, :],
                        scalar1=A[:, h:h + 1], scalar2=B[:, h:h + 1],
                        op0=OP.mult, op1=OP.add)
                    nc.vector.tensor_scalar(
                        out=reg[:, h, :], in0=reg[:, h, :],
                        scalar1=0.0, scalar2=None, op0=OP.max)
                nc.sync.dma_start(out=ytd[...], in_=reg[...])

            # ============ phase 6/9: GAT x@W -> table + hd ============
            def gat_mm(regs, W_t, wd_t, tab_in, hd_dr):
                off = 0
                for reg, n_loc in regs:
                    for t in range(-(-n_loc // P)):
                        m = min(P, n_loc - t * P)
                        pm = pp.tile([P, F + 1], F32, tag="bigps", space="PSUM")
                        ph = pp.tile([1, 512], F32, tag="rowps", space="PSUM")
                        for h in range(2):
                            nc.tensor.matmul(
                                out=pm[:m, :], lhsT=reg[:, h, t * P:t * P + m],
                                rhs=W_t[:, h, :], start=(h == 0), stop=(h == 1))
                        for h in range(2):
                            nc.tensor.matmul(
                                out=ph[:1, :m], lhsT=wd_t[:, h, :],
                                rhs=reg[:, h, t * P:t * P + m],
                                start=(h == 0), stop=(h == 1))
                        sb = wrk.tile([P, TABW], BF16, tag="tabsb")
                        nc.vector.memset(sb[:, 0:1], 1.0)
                        nc.vector.memset(sb[:, F + 2:], 0.0)
                        nc.scalar.activation(out=sb[:m, 1:F + 2],
                                             in_=pm[:m, 0:F + 1], func=AF.Copy)
                        hsb = sm.tile([1, P], BF16, tag="hdsb")
                        nc.vector.tensor_copy(out=hsb[:, :m], in_=ph[:1, :m])
                        nc.sync.dma_start(
                            out=tab_in[off + t * P:off + t * P + m, :],
                            in_=sb[:m, :])
                        nc.sync.dma_start(
                            out=hd_dr[:, off + t * P:off + t * P + m],
                            in_=hsb[:, :m])
                    off += n_loc

            gat_mm([(bn_ab, nl_ab), (bn_ag, nl_ag)], W1_t, wd1_t, tab1_in, hd1_dr)
            nc.gpsimd.collective_compute(
                "AllGather", OP.bypass, replica_groups=rg,
                ins=[tab1_in[...].opt()], outs=[tab1[...].opt()])

            # ============ phase 8/11: GAT aggregation ============
            def gat_agg(wk_list, tab, tab_in, hd_dr, dest_regs, relu_bias):
                n_win = len(wk_list)
                idx_off = 0
                mcol_off = 0
                for w in range(n_win):
                    ks = wk_list[w]
                    K = 1 + sum(ks)
                    mw = min(P, nl_g - w * P)
                    gt = gpool.tile([P, kmax_gat, TABW], BF16, tag="gbuf")
                    if mw < P:
                        nc.vector.memset(gt[:, 0, :], 0.0)
                    nc.sync.dma_start(
                        out=gt[:mw, 0, :],
                        in_=tab_in[w * P:w * P + mw, :])
                    co = 1
                    for h, k in enumerate(ks):
                        if k == 0:
                            continue
                        src_ap = tab[...] if h == 0 else tab[I16_SPLIT:, :]
                        for a in range(0, k, 8):
                            kk = min(8, k - a)
                            nc.gpsimd.dma_gather(
                                out_ap=gt[:, co + a:co + a + kk, :],
                                in_ap=src_ap,
                                idxs_ap=gidx["gg"][:, idx_off + a * 8:
                                                   idx_off + (a + kk) * 8],
                                num_idxs=kk * P, num_idxs_reg=kk * P,
                                elem_size=TABW)
                        idx_off += k * 8
                        co += k
                    mk = mpool.tile([P, kmax_gat, P], BF16, tag="mkbuf")
                    nc.sync.dma_start(
                        out=mk[:, :K, :],
                        in_=ins["gg_mask"][:, mcol_off:mcol_off + K * P])
                    mcol_off += K * P
                    # hd broadcast for this window
                    hdrow = sm.tile([1, P], BF16, tag="hdrow")
                    nc.vector.memset(hdrow[...], 0.0)
                    nc.sync.dma_start(out=hdrow[:, :mw],
                                      in_=hd_dr[:, w * P:w * P + mw])
                    phd = pp.tile([P, P], F32, tag="hdbc", space="PSUM")
                    nc.tensor.matmul(out=phd[...], lhsT=ones_row_t[...],
                                     rhs=hdrow[...], start=True, stop=True)
                    phd_sb = sm.tile([P, P], BF16, tag="phdsb")
                    nc.vector.tensor_copy(out=phd_sb[...], in_=phd[...])
                    # logits strip: t2 = mask * hs_bcast + hd_bcast
                    t2 = spool.tile([P, kmax_gat, P], BF16, tag="t2")
                    nc.vector.tensor_tensor(
                        out=t2[:, :K, :], in0=mk[:, :K, :],
                        in1=gt[:, :K, F + 1:F + 2].broadcast_to([P, K, P]),
                        op=OP.mult)
                    nc.vector.tensor_tensor(
                        out=t2[:, :K, :], in0=t2[:, :K, :],
                        in1=phd_sb[...].unsqueeze(1).broadcast_to([P, K, P]),
                        op=OP.add)
                    # leaky relu (max(x, 0.2x)) then exp, then mask
                    nc.vector.scalar_tensor_tensor(
                        out=t2[:, :K, :], in0=t2[:, :K, :], scalar=0.2,
                        in1=t2[:, :K, :], op0=OP.mult, op1=OP.max)
                    nc.scalar.activation(out=t2[:, :K, :], in_=t2[:, :K, :],
                                         func=AF.Exp)
                    ust = spool.tile([P, kmax_gat, P], BF16, tag="ustr")
                    nc.vector.tensor_tensor(
                        out=ust[:, :K, :], in0=t2[:, :K, :], in1=mk[:, :K, :],
                        op=OP.mult)
                    pm = pp.tile([P, F + 1], F32, tag="bigps", space="PSUM")
                    for c in range(K):
                        nc.tensor.matmul(
                            out=pm[...], lhsT=ust[:, c, :],
                            rhs=gt[:, c, 0:F + 1],
                            start=(c == 0), stop=(c == K - 1))
                    den = sm.tile([P, 1], F32, tag="den")
                    nc.vector.tensor_scalar(out=den[...], in0=pm[:, 0:1],
                                            scalar1=1e-30, scalar2=None,
                                            op0=OP.add)
                    rcp = sm.tile([P, 1], F32, tag="rcp")
                    nc.vector.reciprocal(out=rcp[...], in_=den[...])
                    xo = wrk.tile([P, F], BF16, tag="xo")
                    nc.vector.tensor_scalar(out=xo[...], in0=pm[:, 1:F + 1],
                                            scalar1=rcp[...], scalar2=None,
                                            op0=OP.mult)
                    if relu_bias:
                        nc.vector.tensor_tensor(out=xo[...], in0=xo[...],
                                                in1=bgat_t[...], op=OP.add)
                        nc.vector.tensor_scalar(out=xo[...], in0=xo[...],
                                                scalar1=0.0, scalar2=None,
                                                op0=OP.max)
                    base = w * P
                    for h in range(2):
                        pt = pp.tile([P, P], BF16, tag="trps", space="PSUM")
                        nc.tensor.transpose(out=pt[...],
                                            in_=xo[:, h * P:(h + 1) * P],
                                            identity=ident_t[...])
                        for reg, pstart, plen in dest_regs:
                            a = max(base, pstart)
                            b = min(base + P, pstart + plen)
                            if a < b:
                                nc.vector.tensor_copy(
                                    out=reg[:, h, a - pstart:b - pstart],
                                    in_=pt[:, a - base:b - base])

            g2_ab = xtp.tile([P, 2, nl_ab], BF16, name="g2_ab", tag="xtreg")
            g2_ag = xtp.tile([P, 2, nl_ag], BF16, name="g2_ag", tag="xtreg")
            gat_agg(st["wk_g"], tab1, tab1_in, hd1_dr,
                    [(g2_ab, 0, nl_ab), (g2_ag, nl_ab, nl_ag)], relu_bias=True)

            gat_mm([(g2_ab, nl_ab), (g2_ag, nl_ag)], W2_t, wd2_t, tab2_in, hd2_dr)
            nc.gpsimd.collective_compute(
                "AllGather", OP.bypass, replica_groups=rg,
                ins=[tab2_in[...].opt()], outs=[tab2[...].opt()])

            x1_ab = xtp.tile([P, 2, nl_ab], BF16, name="x1_ab", tag="xtreg")
            x1_ag = xtp.tile([P, 2, nl_ag], BF16, name="x1_ag", tag="xtreg")
            gat_agg(st["wk_g"], tab2, tab2_in, hd2_dr,
                    [(x1_ab, 0, nl_ab), (x1_ag, nl_ab, nl_ag)], relu_bias=False)

            # ============ phase 12: BN2 + FC ============
            bn2_sb = sm.tile([P, 16], F32, bufs=1)
            for si, (x1reg, yt_dr, n_loc) in enumerate(
                    [(x1_ab, yt_ab_dr, nl_ab), (x1_ag, yt_ag_dr, nl_ag)]):
                for ft in range(4):
                    if ft < 2:
                        src = x1reg[:, ft, :]
                    else:
                        yt = wrk.tile([P, n_loc], BF16, tag="ytld", bufs=2)
                        nc.sync.dma_start(out=yt[...], in_=yt_dr[:, ft - 2, :])
                        src = yt[...]
                    col = si * 8 + ft * 2
                    sqt = wrk.tile([P, n_loc], F32, tag="sq2", bufs=1)
                    nc.scalar.activation(out=sqt[...], in_=src, func=AF.Copy,
                                         accum_out=bn2_sb[:, col:col + 1])
                    nc.scalar.activation(out=sqt[...], in_=src, func=AF.Square,
                                         accum_out=bn2_sb[:, col + 1:col + 2])

            nc.sync.dma_start(out=bn2_in[...], in_=bn2_sb[...])
            nc.gpsimd.collective_compute(
                "AllReduce", OP.add, replica_groups=rg,
                ins=[bn2_in[...].opt()], outs=[bn2_out[...].opt()])
            bn2_red = sm.tile([P, 16], F32, bufs=1)
            nc.sync.dma_start(out=bn2_red[...], in_=bn2_out[...])

            for si, (x1reg, yt_dr, gk, bek, wt, bconst, outp, n_loc) in enumerate([
                    (x1_ab, yt_ab_dr, "g2c", "be2c", wfc_t, sc["bfc"], out_ab,
                     nl_ab),
                    (x1_ag, yt_ag_dr, "agg2c", "agbe2c", wagfc_t, sc["bagfc"],
                     out_ag, nl_ag)]):
                A, B = bn_coeffs(bn2_red[:, si * 8:si * 8 + 8:2],
                                 bn2_red[:, si * 8 + 1:si * 8 + 8:2],
                                 bn2cols[gk], bn2cols[bek], 4, "b2")
                ftiles = []
                for ft in range(4):
                    if ft < 2:
                        src = x1reg[:, ft, :]
                    else:
                        yt = wrk.tile([P, n_loc], BF16, tag="ytld2", bufs=2)
                        nc.sync.dma_start(out=yt[...], in_=yt_dr[:, ft - 2, :])
                        src = yt[...]
                    nc.vector.tensor_scalar(
                        out=src, in0=src,
                        scalar1=A[:, ft:ft + 1], scalar2=B[:, ft:ft + 1],
                        op0=OP.mult, op1=OP.add)
                    nc.vector.tensor_scalar(
                        out=src, in0=src,
                        scalar1=0.0, scalar2=None, op0=OP.max)
                    ftiles.append(src)
                for s0 in range(0, n_loc, 512):
                    m = min(512, n_loc - s0)
                    pf = pp.tile([1, 512], F32, tag="rowps", space="PSUM")
                    for ft in range(4):
                        nc.tensor.matmul(
                            out=pf[:1, :m], lhsT=wt[:, ft:ft + 1],
                            rhs=ftiles[ft][:, s0:s0 + m],
                            start=(ft == 0), stop=(ft == 3))
                    ob = sm.tile([1, 512], F32, tag="fcsb")
                    nc.vector.tensor_scalar(out=ob[:, :m], in0=pf[:1, :m],
                                            scalar1=bconst, scalar2=None,
                                            op0=OP.add)
                    nc.sync.dma_start(out=outp[:, s0:s0 + m], in_=ob[:, :m])

    nc.finalize()
    return nc


# ----------------------------------------------------------------------------
# runner
# ----------------------------------------------------------------------------

_CACHE = {}


def _run(inputs, n_ab, n_ag, trace=False, sim=False):
    static, in_maps = build_host_plan(inputs, n_ab, n_ag, CORES)
    key = (n_ab, n_ag,
           hash(np.asarray(inputs["edge_index_d"]).tobytes()) ^
           hash(np.asarray(inputs["edge_x_ab"]).tobytes()) ^
           hash(np.asarray(inputs["edge_x_ag"]).tobytes()) ^
           hash(repr(sorted(static["scalars"].items()))))
    if key not in _CACHE:
        _CACHE[key] = build_bass(static)
    nc = _CACHE[key]
    nl_ab, nl_ag = n_ab // CORES, n_ag // CORES

    if sim:
        from concourse import bass_interp
        s = bass_interp.MultiCoreSim(nc, CORES)
        for i in range(CORES):
            for k, v in in_maps[i].items():
                s.cores[i].tensor(k)[:] = v
        s.simulate()
        o_ab = np.concatenate(
            [s.cores[c].mem_tensor("out_ab").reshape(nl_ab, 1)
             for c in range(CORES)], 0)
        o_ag = np.concatenate(
            [s.cores[c].mem_tensor("out_ag").reshape(nl_ag, 1)
             for c in range(CORES)], 0)
        return (o_ab, o_ag), None

    from concourse.bass_utils import run_bass_kernel_spmd
    r = run_bass_kernel_spmd(nc, in_maps, core_ids=list(range(CORES)),
                             trace=trace)
    o_ab = np.concatenate(
        [r.results[c]["out_ab"].reshape(nl_ab, 1) for c in range(CORES)], 0)
    o_ag = np.concatenate(
        [r.results[c]["out_ag"].reshape(nl_ag, 1) for c in range(CORES)], 0)
    return (o_ab, o_ag), r


def kernel(**inputs):
    (o_ab, o_ag), _ = _run(inputs, 20000, 20000)
    return o_ab, o_ag
